# revision 58
# baseline (speedup 1.0000x reference)
"""Trainium2 Bass kernel for nn_HCF_module (SC2 NMS/registration pipeline).

Single fused device launch (512 seeds sharded 64/core over 8 NeuronCores,
keypoints replicated). Host does only the exact top-200 seed-row selection
(stable argsort = lax.top_k tie order), ships one packed [64, 392] input
per core (200 indices + keypoint scatter rows), and does the final
argmax/T assembly. One input + one output dram tensor per core — per-array
RPC overhead (~40ms/array) dominates the launch wall on axon.

Device program per core (64 seeds on 64 partitions):
  - replicate keypoints to all partitions (scatter rows -> Internal DRAM
    -> stride-0 broadcast DMA), then gather each seed's top-200 coords
    via exact one-hot is_eq over a device-built 0..2047 iota.
  - 4 filter stages k=200/100/50/25: SC2 consistency scores (sqrt-free
    hard-bit test, bit-identical to the validated baseline arithmetic),
    then EXACT top-k/2 selection via unique integer keys 256*sc2 - pos
    (f32-exact integers; DVE max8/match_replace rounds), then one-hot
    is_eq gather of the selected neighbor coords (exact f32 copies).
  - tail: local_sc matrix, 10x power iteration, weighted Kabsch via
    closed-form 3x3 eigensolver + Newton (same op order as the validated
    f32 host model), fitness inlier counts over all 2048 keypoints.
Outputs per seed (packed [64, 13]): inlier count, R (3x3), t (3).
"""
import os as _os
import numpy as np

# Persistent XLA compilation cache: the PJRT wrapper is re-traced per launch
# (fresh closure inside run_bass_kernel_spmd), so without this every warm
# launch re-runs the BIR->NEFF backend pipeline (~0.3s). Must be set before
# jax initializes.
_os.environ.setdefault("JAX_COMPILATION_CACHE_DIR", "/tmp/jax_comp_cache")
_os.environ.setdefault("JAX_PERSISTENT_CACHE_MIN_COMPILE_TIME_SECS", "0")
_os.environ.setdefault("JAX_PERSISTENT_CACHE_MIN_ENTRY_SIZE_BYTES", "0")

F32 = np.float32
T2 = F32(0.1) * F32(0.1)            # 0.010000000707...
TWO_T2 = F32(2.0) * T2
T4 = T2 * T2
INV_T2 = F32(np.float64(1.0) / np.float64(T2))
NCORES = 8
SEEDS = 512
SPC = SEEDS // NCORES               # seeds per core
NPTS = 2048
NEG = -1e30

# filter stages: (k, B, kf, gather-chunk mc)
STAGES = [(200, 10, 100, 25), (100, 20, 50, 50), (50, 25, 25, 25), (25, 25, 12, 12)]

_programs = {}
_launch_wall = []


def _mk_bass(detect_races=True):
    import concourse.bass as bass
    return bass.Bass("TRN2", target_bir_lowering=False,
                     detect_race_conditions=detect_races)


def _prog_mega(debug=False, sync_all=True, trunc=0):
    """Build the fused device program.

    sync_all=True emits a vsem inc+wait after every DVE instruction —
    required by CoreSim's race model (used for validation builds).
    sync_all=False relies on in-order engine execution with the HW's
    per-op pipeline drain, fencing only at ACT/DMA crossings (faster).
    """
    import concourse.mybir as mybir
    from concourse.alu_op_type import AluOpType as OP
    nc = _mk_bass(detect_races=sync_all)
    P = SPC
    # single packed input per core: [0:200) top-200 knn indices (f32 integers)
    # | [200:392) keypts scatter (row r holds pts.flat[r*192:(r+1)*192],
    # pts.flat = src c-major 6144 floats then tgt c-major 6144 floats)
    inp = nc.dram_tensor("inp", [P, 392], mybir.dt.float32, kind="ExternalInput")
    dscr = nc.dram_tensor("dscr", [1, 2 * 3 * NPTS], mybir.dt.float32, kind="Internal")
    # single packed output: col 0 cnt | 1:10 R row-major | 10:13 t
    out13 = nc.dram_tensor("out13", [P, 13], mybir.dt.float32, kind="ExternalOutput")
    dbg_names = []
    if debug:
        dbg_specs = [("dsc1", 200), ("dsc2", 100), ("dsc3", 50), ("dsc4", 25),
                     ("dx2", 300), ("dxf", 36), ("dyf", 36), ("dm", 144),
                     ("dvv", 12), ("dww", 12), ("dh9", 9), ("dk9", 9),
                     ("dlam", 2), ("du1", 3), ("du2", 3), ("dv1", 3)]
        dbg_dram = {n: nc.dram_tensor(n, [P, w], mybir.dt.float32, kind="ExternalOutput")
                    for (n, w) in dbg_specs}
        dbg_names = [n for (n, _) in dbg_specs]

    ctx = nc.ctx
    sb = lambda nm, shape: ctx.enter_context(nc.sbuf_tensor(nm, shape, mybir.dt.float32))[:, :]
    INP = sb("INP", [P, 392])
    IDX = INP[:, 0:200]
    POSI = ctx.enter_context(nc.sbuf_tensor("POSI", [P, 200], mybir.dt.int32))[:, :]
    POS = sb("POS", [P, 200])
    TXa = sb("TXa", [P, 600]); TYa = sb("TYa", [P, 600])
    TXb = sb("TXb", [P, 304]); TYb = sb("TYb", [P, 304])
    TXc = sb("TXc", [P, 304]); TYc = sb("TYc", [P, 304])
    SC2S = sb("SC2S", [P, 200]); H0 = sb("H0", [P, 200])
    KEYP = sb("KEYP", [P, 200]); KEYW = sb("KEYW", [P, 200]); TOPV = sb("TOPV", [P, 104])
    IOTA2K = sb("IOTA2K", [P, NPTS])
    PSRC = sb("PSRC", [P, 3 * NPTS]); PTGT = sb("PTGT", [P, 3 * NPTS])
    VV = sb("VV", [P, 12]); WW = sb("WW", [P, 12])
    OUT13 = sb("OUT13", [P, 13])
    CNTS = OUT13[:, 0:1]; R9S = OUT13[:, 1:10]; T3S = OUT13[:, 10:13]
    FEN = sb("FEN", [P, 1])
    SCR = sb("SCR", [P, 25000])
    if debug:
        dbg_sb = {n: sb("sb_" + n, [P, w]) for (n, w) in dbg_specs}

    dins = ctx.enter_context(nc.semaphore())
    dpts = ctx.enter_context(nc.semaphore())
    dout = ctx.enter_context(nc.semaphore())
    vsem = ctx.enter_context(nc.semaphore())
    asem = ctx.enter_context(nc.semaphore())
    gsem = ctx.enter_context(nc.semaphore())

    vcnt = [0]
    acnt = [0]
    sqrt_jobs = []   # (vsem threshold, src AP, dst AP)
    veng = [None]
    marks = {}

    def V(inst):
        if sync_all:
            inst.then_inc(vsem, 1)
            vcnt[0] += 1
            veng[0].wait_ge(vsem, vcnt[0])
        return inst

    def fence():
        # make vsem reflect completion of all vector work so far
        if not sync_all:
            nc.vector.tensor_copy(FEN, FEN).then_inc(vsem, 1)
            vcnt[0] += 1

    def tt(out, a, b, op):
        V(nc.vector.tensor_tensor(out=out, in0=a, in1=b, op=op))

    def ts(out, a, s1, op0, s2=None, op1=None):
        if op1 is None:
            V(nc.vector.tensor_scalar(out, a, s1, None, op0))
        else:
            V(nc.vector.tensor_scalar(out, a, s1, s2, op0, op1))

    def stt(out, in0, s, in1, op0, op1):
        V(nc.vector.scalar_tensor_tensor(out=out, in0=in0, scalar=s, in1=in1,
                                         op0=op0, op1=op1))

    def cp(out, a):
        V(nc.vector.tensor_copy(out, a))

    def red(out, in_, op=None):
        V(nc.vector.tensor_reduce(out=out, in_=in_, axis=mybir.AxisListType.X,
                                  op=op or OP.add))

    def mset(ap, v):
        V(nc.vector.memset(ap, v))

    def rcp(out, in_):
        V(nc.vector.reciprocal(out, in_))

    def act_sqrt(dst, src):
        fence()
        sqrt_jobs.append((vcnt[0], src, dst))
        acnt[0] += 1
        veng[0].wait_ge(asem, acnt[0])

    def sc2_stage(k, B, tx, ty):
        dxs = SCR[:, 0:B * 3 * k]
        d2a = SCR[:, 6000:6000 + B * k]
        d2b = SCR[:, 8000:8000 + B * k]
        q = SCR[:, 10000:10000 + B * k]
        pp = SCR[:, 12000:12000 + B * k]
        hard = SCR[:, 14000:14000 + B * k]
        scr2 = SCR[:, 16000:16000 + B * k]
        nb = k // B
        for bi in range(nb):
            a0 = bi * B
            for (src_t, dst) in ((tx, d2a), (ty, d2b)):
                v3 = src_t[:, :3 * k].rearrange("p (c b) -> p c b", c=3)
                rows4 = v3.unsqueeze(1).to_broadcast([P, B, 3, k])
                cols4 = v3[:, :, a0:a0 + B].transpose([0, 2, 1]).unsqueeze(3).to_broadcast([P, B, 3, k])
                dx4 = dxs.rearrange("p (a c b) -> p a c b", a=B, c=3)
                tt(dx4, rows4, cols4, OP.subtract)
                tt(dxs, dxs, dxs, OP.mult)
                d2v = dst.rearrange("p (a b) -> p a b", a=B)
                tt(d2v, dx4[:, :, 0, :], dx4[:, :, 1, :], OP.add)
                tt(d2v, d2v, dx4[:, :, 2, :], OP.add)
            tt(q, d2a, d2b, OP.add)
            tt(pp, d2a, d2b, OP.subtract)
            tt(pp, pp, pp, OP.mult)
            ts(scr2, q, float(TWO_T2), OP.mult, float(T4), OP.subtract)
            tt(hard, pp, scr2, OP.is_lt)
            ts(scr2, q, float(T2), OP.is_lt)
            tt(hard, hard, scr2, OP.max)
            if bi == 0:
                cp(H0[:, :k], hard[:, :k])
            hv = hard.rearrange("p (a b) -> p a b", a=B)
            h0c = H0[:, a0:a0 + B].unsqueeze(2).to_broadcast([P, B, k])
            tt(hv, hv, h0c, OP.mult)
            hT = hv.transpose([0, 2, 1])
            if bi == 0:
                red(SC2S[:, :k], hT)
            else:
                red(scr2[:, :k], hT)
                tt(SC2S[:, :k], SC2S[:, :k], scr2[:, :k], OP.add)

    def key_topk(k, kf):
        # unique integer keys: 256*sc2 - pos; desc key order == (sc2 desc, pos asc)
        ts(KEYP[:, :k], SC2S[:, :k], 256.0, OP.mult)
        tt(KEYP[:, :k], KEYP[:, :k], POS[:, :k], OP.subtract)
        cp(KEYW[:, :k], KEYP[:, :k])
        rounds = (kf + 7) // 8
        for r in range(rounds):
            V(nc.vector.max(out=TOPV[:, r * 8:(r + 1) * 8], in_=KEYW[:, :k]))
            if r < rounds - 1:
                V(nc.vector.match_replace(out=KEYW[:, :k],
                                          in_to_replace=TOPV[:, r * 8:(r + 1) * 8],
                                          in_values=KEYW[:, :k], imm_value=NEG))

    def gather_top200():
        # TXa/TYa[:, c*200+m] = keypts[idx[m], c] via exact one-hot over 2048
        cp(POS, POSI)   # int32 -> f32, exact for 0..199 (iota runs on gpsimd)
        for c in range(10):
            ts(IOTA2K[:, c * 200:(c + 1) * 200], POS, float(200 * c), OP.add)
        ts(IOTA2K[:, 2000:2048], POS[:, 0:48], 2000.0, OP.add)
        mcg = 6
        pv3 = PSRC.rearrange("p (c n) -> p c n", c=3)
        tv3 = PTGT.rearrange("p (c n) -> p c n", c=3)
        for c0 in range(0, 200, mcg):
            w = min(mcg, 200 - c0)
            oh3 = SCR[:, 0:w * NPTS].rearrange("p (m j) -> p m j", m=w)
            tmp3 = SCR[:, mcg * NPTS:mcg * NPTS + w * NPTS].rearrange("p (m j) -> p m j", m=w)
            sel = IDX[:, c0:c0 + w]
            tt(oh3, sel.unsqueeze(2).to_broadcast([P, w, NPTS]),
               IOTA2K.unsqueeze(1).to_broadcast([P, w, NPTS]), OP.is_equal)
            for (src3, t_out) in ((pv3, TXa), (tv3, TYa)):
                for c in range(3):
                    tt(tmp3, oh3,
                       src3[:, c, :].unsqueeze(1).to_broadcast([P, w, NPTS]),
                       OP.mult)
                    red(t_out[:, c * 200 + c0:c * 200 + c0 + w], tmp3)

    def gather(k, kf, mc, tx, ty, ox, oy):
        oh3 = SCR[:, 0:mc * k].rearrange("p (m j) -> p m j", m=mc)
        tmp3 = SCR[:, mc * k:2 * mc * k].rearrange("p (m j) -> p m j", m=mc)
        for c0 in range(0, kf, mc):
            sel = TOPV[:, c0:c0 + mc]
            tt(oh3, sel.unsqueeze(2).to_broadcast([P, mc, k]),
               KEYP[:, :k].unsqueeze(1).to_broadcast([P, mc, k]), OP.is_equal)
            for (t_in, t_out) in ((tx, ox), (ty, oy)):
                for c in range(3):
                    tt(tmp3, oh3,
                       t_in[:, c * k:(c + 1) * k].unsqueeze(1).to_broadcast([P, mc, k]),
                       OP.mult)
                    red(t_out[:, c * kf + c0:c * kf + c0 + mc], tmp3)

    scr_off = [0]

    def alloc(n):
        off = scr_off[0]
        scr_off[0] += n
        assert scr_off[0] <= 12000
        return SCR[:, off:off + n]

    def cross3(out, a, b, tA, tB):
        for i in range(3):
            j, kk = (i + 1) % 3, (i + 2) % 3
            tt(tA, a[:, j:j + 1], b[:, kk:kk + 1], OP.mult)
            tt(tB, a[:, kk:kk + 1], b[:, j:j + 1], OP.mult)
            tt(out[:, i:i + 1], tA, tB, OP.subtract)

    def normalize3(u, nu, ns, rn, t3v, eps=1e-38):
        # u *= 1/sqrt(max(sum(u^2), eps))
        tt(t3v, u, u, OP.mult)
        red(nu, t3v)
        ts(nu, nu, eps, OP.max)
        act_sqrt(ns, nu)
        rcp(rn, ns)
        ts(u, u, rn, OP.mult)

    with nc.Block() as block:
        @block.vector
        def _(vector):
            veng[0] = vector
            mset(FEN, 0.0)
            vector.wait_ge(dins, 16)     # INP DMA
            vector.wait_ge(gsem, 1)      # gpsimd iota
            vector.wait_ge(dpts, 48)     # PSRC/PTGT replicated
            gather_top200()
            curx, cury = TXa, TYa
            for si, (k, B, kf, mc) in enumerate(STAGES):
                nxtx, nxty = (TXb, TYb) if si % 2 == 0 else (TXc, TYc)
                sc2_stage(k, B, curx, cury)
                if trunc == 1 and si == 0:
                    fence()
                    return
                if debug:
                    cp(dbg_sb[["dsc1", "dsc2", "dsc3", "dsc4"][si]], SC2S[:, :k])
                key_topk(k, kf)
                gather(k, kf, mc, curx, cury, nxtx, nxty)
                if debug and si == 0:
                    cp(dbg_sb["dx2"], nxtx[:, :300])
                curx, cury = nxtx, nxty
            if trunc == 2:
                fence()
                return
            # final selected coords: curx[:, :36], cury[:, :36] (c-major, 12 each)
            if debug:
                cp(dbg_sb["dxf"], curx[:, :36])
                cp(dbg_sb["dyf"], cury[:, :36])

            # ---- local_sc matrix M [12x12] ----
            DX = alloc(432)
            A2 = alloc(144); B2 = alloc(144)
            DA = alloc(144); DB = alloc(144)
            CR = alloc(144); M144 = alloc(144); PR = alloc(144)
            for (t_in, d2out) in ((curx, A2), (cury, B2)):
                v3 = t_in[:, :36].rearrange("p (c b) -> p c b", c=3)
                rows4 = v3.unsqueeze(1).to_broadcast([P, 12, 3, 12])
                cols4 = v3.transpose([0, 2, 1]).unsqueeze(3).to_broadcast([P, 12, 3, 12])
                dx4 = DX.rearrange("p (a c b) -> p a c b", a=12, c=3)
                tt(dx4, rows4, cols4, OP.subtract)
                tt(DX, DX, DX, OP.mult)
                d2v = d2out.rearrange("p (a b) -> p a b", a=12)
                tt(d2v, dx4[:, :, 0, :], dx4[:, :, 1, :], OP.add)
                tt(d2v, d2v, dx4[:, :, 2, :], OP.add)
            ts(A2, A2, 1e-12, OP.max)
            ts(B2, B2, 1e-12, OP.max)
            act_sqrt(DA, A2)
            act_sqrt(DB, B2)
            tt(CR, DA, DB, OP.subtract)
            tt(CR, CR, CR, OP.mult)   # |da-db|^2 == (da-db)^2 exactly
            ts(M144, CR, -float(INV_T2), OP.mult, 1.0, OP.add)
            ts(M144, M144, 0.0, OP.max)
            for i in range(12):
                mset(M144[:, 13 * i:13 * i + 1], 0.0)
            if debug:
                cp(dbg_sb["dm"], M144)

            # ---- power iteration (10 iters) ----
            m3 = M144.rearrange("p (i j) -> p i j", i=12)
            VN = alloc(12); T12 = alloc(12)
            N2 = alloc(1); NN = alloc(1); RN = alloc(1)
            mset(VV, 1.0)
            for _ in range(10):
                tt(PR.rearrange("p (i j) -> p i j", i=12), m3,
                   VV.unsqueeze(1).to_broadcast([P, 12, 12]), OP.mult)
                red(VN, PR.rearrange("p (i j) -> p i j", i=12))
                tt(T12, VN, VN, OP.mult)
                red(N2, T12)
                act_sqrt(NN, N2)
                ts(NN, NN, 1e-6, OP.add)
                rcp(RN, NN)
                ts(VV, VN, RN, OP.mult)
            if debug:
                cp(dbg_sb["dvv"], VV)
            # w = v / (sum(v) + 1e-6)
            S1 = alloc(1); RS = alloc(1)
            red(S1, VV)
            ts(S1, S1, 1e-6, OP.add)
            rcp(RS, S1)
            ts(WW, VV, RS, OP.mult)
            if debug:
                cp(dbg_sb["dww"], WW)

            # ---- weighted Kabsch ----
            a3 = curx[:, :36].rearrange("p (c b) -> p c b", c=3)
            b3 = cury[:, :36].rearrange("p (c b) -> p c b", c=3)
            WS = alloc(1); RWS = alloc(1)
            red(WS, WW)
            ts(WS, WS, 1e-6, OP.add)
            rcp(RWS, WS)
            WA = alloc(36); SA = alloc(3); CA = alloc(3); CB = alloc(3)
            AM = alloc(36); BM = alloc(36); WAM = alloc(36)
            wb = WW.unsqueeze(1).to_broadcast([P, 3, 12])
            tt(WA.rearrange("p (c b) -> p c b", c=3), a3, wb, OP.mult)
            red(SA, WA.rearrange("p (c b) -> p c b", c=3))
            ts(CA, SA, RWS, OP.mult)
            tt(WA.rearrange("p (c b) -> p c b", c=3), b3, wb, OP.mult)
            red(SA, WA.rearrange("p (c b) -> p c b", c=3))
            ts(CB, SA, RWS, OP.mult)
            tt(AM.rearrange("p (c b) -> p c b", c=3), a3,
               CA.unsqueeze(2).to_broadcast([P, 3, 12]), OP.subtract)
            tt(BM.rearrange("p (c b) -> p c b", c=3), b3,
               CB.unsqueeze(2).to_broadcast([P, 3, 12]), OP.subtract)
            tt(WAM.rearrange("p (c b) -> p c b", c=3),
               AM.rearrange("p (c b) -> p c b", c=3), wb, OP.mult)
            HP = alloc(108); H9 = alloc(9)
            tt(HP.rearrange("p (i j b) -> p i j b", i=3, j=3),
               WAM.rearrange("p (c b) -> p c b", c=3).unsqueeze(2).to_broadcast([P, 3, 3, 12]),
               BM.rearrange("p (c b) -> p c b", c=3).unsqueeze(1).to_broadcast([P, 3, 3, 12]),
               OP.mult)
            red(H9, HP.rearrange("p (i j b) -> p i j b", i=3, j=3))
            if debug:
                cp(dbg_sb["dh9"], H9)
            KP = alloc(27); K9 = alloc(9)
            h3v = H9.rearrange("p (i j) -> p i j", i=3)
            tt(KP.rearrange("p (i l j) -> p i l j", i=3, l=3),
               h3v.unsqueeze(2).to_broadcast([P, 3, 3, 3]),
               h3v.unsqueeze(1).to_broadcast([P, 3, 3, 3]), OP.mult)
            red(K9, KP.rearrange("p (i l j) -> p i l j", i=3, l=3))
            if debug:
                cp(dbg_sb["dk9"], K9)

            # ---- closed-form eigenvalues of K (3x3 sym PSD) ----
            c1_ = lambda i: K9[:, i:i + 1]
            QQ = alloc(1)
            tt(QQ, c1_(0), c1_(4), OP.add)
            tt(QQ, QQ, c1_(8), OP.add)
            ts(QQ, QQ, float(F32(1.0 / 3.0)), OP.mult)
            KD = alloc(3)   # K00-qq, K11-qq, K22-qq
            for di, src_i in enumerate((0, 4, 8)):
                tt(KD[:, di:di + 1], c1_(src_i), QQ, OP.subtract)
            P1 = alloc(1); TTa = alloc(1); TTb = alloc(1)
            tt(P1, c1_(1), c1_(1), OP.mult)
            tt(TTa, c1_(2), c1_(2), OP.mult)
            tt(P1, P1, TTa, OP.add)
            tt(TTa, c1_(5), c1_(5), OP.mult)
            tt(P1, P1, TTa, OP.add)
            P2 = alloc(1)
            tt(P2, KD[:, 0:1], KD[:, 0:1], OP.mult)
            tt(TTa, KD[:, 1:2], KD[:, 1:2], OP.mult)
            tt(P2, P2, TTa, OP.add)
            tt(TTa, KD[:, 2:3], KD[:, 2:3], OP.mult)
            tt(P2, P2, TTa, OP.add)
            ts(TTa, P1, 2.0, OP.mult)
            tt(P2, P2, TTa, OP.add)
            PV = alloc(1); RP = alloc(1)
            ts(PV, P2, float(F32(1.0 / 6.0)), OP.mult)
            act_sqrt(PV, PV)
            ts(TTa, PV, 1e-30, OP.max)
            rcp(RP, TTa)
            BV = alloc(6)   # B00,B11,B22,B01,B02,B12
            for bi_, src in enumerate((KD[:, 0:1], KD[:, 1:2], KD[:, 2:3],
                                       c1_(1), c1_(2), c1_(5))):
                ts(BV[:, bi_:bi_ + 1], src, RP, OP.mult)
            B00, B11, B22 = BV[:, 0:1], BV[:, 1:2], BV[:, 2:3]
            B01, B02, B12 = BV[:, 3:4], BV[:, 4:5], BV[:, 5:6]
            DET = alloc(1); TTc = alloc(1)
            # t1 = B00*(B11*B22 - B12*B12)
            tt(TTa, B11, B22, OP.mult)
            tt(TTb, B12, B12, OP.mult)
            tt(TTa, TTa, TTb, OP.subtract)
            tt(DET, B00, TTa, OP.mult)
            # t2 = B01*(B01*B22 - B12*B02); det = t1 - t2
            tt(TTa, B01, B22, OP.mult)
            tt(TTb, B12, B02, OP.mult)
            tt(TTa, TTa, TTb, OP.subtract)
            tt(TTc, B01, TTa, OP.mult)
            tt(DET, DET, TTc, OP.subtract)
            # t3 = B02*(B01*B12 - B11*B02); det = det + t3
            tt(TTa, B01, B12, OP.mult)
            tt(TTb, B11, B02, OP.mult)
            tt(TTa, TTa, TTb, OP.subtract)
            tt(TTc, B02, TTa, OP.mult)
            tt(DET, DET, TTc, OP.add)
            RV = alloc(1)
            ts(RV, DET, 0.5, OP.mult)
            ts(RV, RV, -1.0, OP.max)
            ts(RV, RV, 1.0, OP.min)
            CC = alloc(1); C2 = alloc(1); C3 = alloc(1)
            FF = alloc(1); FP = alloc(1); RFP = alloc(1)
            mset(CC, 1.0)
            for _ in range(6):
                tt(C2, CC, CC, OP.mult)
                tt(C3, C2, CC, OP.mult)
                ts(FF, C3, 4.0, OP.mult)
                ts(TTa, CC, 3.0, OP.mult)
                tt(FF, FF, TTa, OP.subtract)
                tt(FF, FF, RV, OP.subtract)
                ts(FP, C2, 12.0, OP.mult, 3.0, OP.subtract)
                ts(FP, FP, 1e-6, OP.max)
                rcp(RFP, FP)
                tt(TTa, FF, RFP, OP.mult)
                tt(CC, CC, TTa, OP.subtract)
                ts(CC, CC, 0.5, OP.max)
                ts(CC, CC, 1.0, OP.min)
            SS = alloc(1)
            tt(SS, CC, CC, OP.mult)
            ts(SS, SS, -1.0, OP.mult, 1.0, OP.add)
            ts(SS, SS, 0.0, OP.max)
            act_sqrt(SS, SS)
            LAM1 = alloc(1); LAM2 = alloc(1)
            ts(TTa, PV, 2.0, OP.mult)
            tt(TTa, TTa, CC, OP.mult)
            tt(LAM1, QQ, TTa, OP.add)
            ts(TTa, CC, -0.5, OP.mult)
            ts(TTb, SS, float(F32(np.sqrt(3.0) / 2.0)), OP.mult)
            tt(TTa, TTa, TTb, OP.add)
            ts(TTb, PV, 2.0, OP.mult)
            tt(TTa, TTa, TTb, OP.mult)
            tt(LAM2, QQ, TTa, OP.add)
            if debug:
                cp(dbg_sb["dlam"][:, 0:1], LAM1)
                cp(dbg_sb["dlam"][:, 1:2], LAM2)

            # ---- eigenvectors ----
            AK = alloc(9)
            C1v = alloc(3); C2v = alloc(3); C3v = alloc(3)
            N1 = alloc(1); N2e = alloc(1); N3e = alloc(1)
            MA = alloc(1); MB = alloc(1); MC = alloc(1)
            T3v = alloc(3); NU = alloc(1); NS = alloc(1); RNU = alloc(1)
            U1 = alloc(3); U2 = alloc(3); U3 = alloc(3)

            def eigvec(lam, uout):
                cp(AK, K9)
                for d in range(3):
                    tt(AK[:, 4 * d:4 * d + 1], AK[:, 4 * d:4 * d + 1], lam, OP.subtract)
                r0, r1, r2 = AK[:, 0:3], AK[:, 3:6], AK[:, 6:9]
                cross3(C1v, r0, r1, TTa, TTb)
                cross3(C2v, r1, r2, TTa, TTb)
                cross3(C3v, r2, r0, TTa, TTb)
                for (cv, nv) in ((C1v, N1), (C2v, N2e), (C3v, N3e)):
                    tt(T3v, cv, cv, OP.mult)
                    red(nv, T3v)
                tt(MA, N1, N2e, OP.is_ge)
                tt(TTa, N1, N3e, OP.is_ge)
                tt(MA, MA, TTa, OP.mult)
                ts(TTa, MA, -1.0, OP.mult, 1.0, OP.add)     # 1 - a1
                tt(MB, N2e, N3e, OP.is_ge)
                tt(MB, TTa, MB, OP.mult)                     # a2
                tt(MC, TTa, MB, OP.subtract)                 # a3
                ts(uout, C1v, MA, OP.mult)
                ts(T3v, C2v, MB, OP.mult)
                tt(uout, uout, T3v, OP.add)
                ts(T3v, C3v, MC, OP.mult)
                tt(uout, uout, T3v, OP.add)
                normalize3(uout, NU, NS, RNU, T3v)

            eigvec(LAM1, U1)
            eigvec(LAM2, U2)
            if debug:
                cp(dbg_sb["du1"], U1)
            # Gram-Schmidt u2 against u1
            DOT = alloc(1)
            tt(T3v, U1, U2, OP.mult)
            red(DOT, T3v)
            ts(T3v, U1, DOT, OP.mult)
            tt(U2, U2, T3v, OP.subtract)
            normalize3(U2, NU, NS, RNU, T3v)
            if debug:
                cp(dbg_sb["du2"], U2)
            cross3(U3, U1, U2, TTa, TTb)

            # v_i = normalize(H^T u_i); v3 = v1 x v2
            HP2 = alloc(9)
            V1 = alloc(3); V2 = alloc(3); V3 = alloc(3)
            ht3 = H9.rearrange("p (i j) -> p i j", i=3).transpose([0, 2, 1])
            for (uin, vout) in ((U1, V1), (U2, V2)):
                tt(HP2.rearrange("p (i j) -> p i j", i=3), ht3,
                   uin.unsqueeze(1).to_broadcast([P, 3, 3]), OP.mult)
                red(vout, HP2.rearrange("p (i j) -> p i j", i=3))
                normalize3(vout, NU, NS, RNU, T3v)
            if debug:
                cp(dbg_sb["dv1"], V1)
            cross3(V3, V1, V2, TTa, TTb)

            # R = v1 u1^T + v2 u2^T + v3 u3^T ;  t = cB - R cA
            OP9 = alloc(9)
            tt(R9S.rearrange("p (i j) -> p i j", i=3),
               V1.unsqueeze(2).to_broadcast([P, 3, 3]),
               U1.unsqueeze(1).to_broadcast([P, 3, 3]), OP.mult)
            for (vv_, uu_) in ((V2, U2), (V3, U3)):
                tt(OP9.rearrange("p (i j) -> p i j", i=3),
                   vv_.unsqueeze(2).to_broadcast([P, 3, 3]),
                   uu_.unsqueeze(1).to_broadcast([P, 3, 3]), OP.mult)
                tt(R9S, R9S, OP9, OP.add)
            tt(OP9.rearrange("p (i j) -> p i j", i=3),
               R9S.rearrange("p (i j) -> p i j", i=3),
               CA.unsqueeze(1).to_broadcast([P, 3, 3]), OP.mult)
            RC = alloc(3)
            red(RC, OP9.rearrange("p (i j) -> p i j", i=3))
            tt(T3S, CB, RC, OP.subtract)

            if trunc == 3:
                fence()
                return
            # ---- fitness: count ||R x + t - y|| < 0.1 over all 2048 pts ----
            DC = SCR[:, 0:6144].rearrange("p (c n) -> p c n", c=3)
            ACC = SCR[:, 6144:6144 + 2048]
            L2S = SCR[:, 8192:8192 + 2048]
            SQ = SCR[:, 10240:10240 + 2048]
            xv = PSRC.rearrange("p (c n) -> p c n", c=3)
            yv = PTGT.rearrange("p (c n) -> p c n", c=3)
            for c in range(3):
                ts(ACC, xv[:, 0, :], R9S[:, 3 * c:3 * c + 1], OP.mult,
                   T3S[:, c:c + 1], OP.add)
                stt(ACC, xv[:, 1, :], R9S[:, 3 * c + 1:3 * c + 2], ACC, OP.mult, OP.add)
                stt(ACC, xv[:, 2, :], R9S[:, 3 * c + 2:3 * c + 3], ACC, OP.mult, OP.add)
                tt(DC[:, c, :], ACC, yv[:, c, :], OP.subtract)
            tt(L2S, DC[:, 0, :], DC[:, 0, :], OP.mult)
            tt(SQ, DC[:, 1, :], DC[:, 1, :], OP.mult)
            tt(L2S, L2S, SQ, OP.add)
            tt(SQ, DC[:, 2, :], DC[:, 2, :], OP.mult)
            tt(L2S, L2S, SQ, OP.add)
            ts(SQ, L2S, float(T2), OP.is_lt)
            red(CNTS, SQ)
            fence()

        @block.scalar
        def _(scalar):
            from concourse import mybir as mb
            for (vt, src, dst) in sqrt_jobs:
                scalar.wait_ge(vsem, vt)
                nc.scalar.sqrt(dst, src).then_inc(asem, 1)

        @block.gpsimd
        def _(gpsimd):
            gpsimd.dma_start(INP, inp[:, :]).then_inc(dins, 16)
            gpsimd.iota(POSI, pattern=[[1, 200]], base=0,
                        channel_multiplier=0).then_inc(gsem, 1)
            # rebuild replicated keypoint rows: scatter -> DRAM -> broadcast
            gpsimd.wait_ge(dins, 16)
            gpsimd.dma_start(dscr[0:1, :].rearrange("p (a b) -> p a b", a=P),
                             INP[:, 200:392]).then_inc(dpts, 16)
            gpsimd.wait_ge(dpts, 16)
            gpsimd.dma_start(PSRC, dscr[0:1, 0:3 * NPTS].to_broadcast([P, 3 * NPTS])).then_inc(dpts, 16)
            gpsimd.dma_start(PTGT, dscr[0:1, 3 * NPTS:6 * NPTS].to_broadcast([P, 3 * NPTS])).then_inc(dpts, 16)
            gpsimd.wait_ge(vsem, vcnt[0])
            nout = 1 + len(dbg_names)
            gpsimd.dma_start(out13[:, :], OUT13).then_inc(dout, 16)
            if debug:
                for n_ in dbg_names:
                    gpsimd.dma_start(dbg_dram[n_][:, :], dbg_sb[n_]).then_inc(dout, 16)
            gpsimd.wait_ge(dout, 16 * nout)
    return nc


def _get_prog(key, builder):
    if key not in _programs:
        _programs[key] = builder()
    return _programs[key]


_cache_cfg = []


def _enable_jax_cache():
    if _cache_cfg:
        return
    _cache_cfg.append(1)
    try:
        import jax
        jax.config.update("jax_compilation_cache_dir", "/tmp/jax_comp_cache")
        jax.config.update("jax_persistent_cache_min_compile_time_secs", 0)
        jax.config.update("jax_persistent_cache_min_entry_size_bytes", 0)
    except Exception:
        pass


def _run(nc, in_maps):
    import time
    _enable_jax_cache()
    from concourse.bass_utils import run_bass_kernel_spmd
    last = None
    for attempt in range(3):
        try:
            t0 = time.time()
            res = run_bass_kernel_spmd(nc, in_maps, core_ids=list(range(NCORES)))
            _launch_wall.append(time.time() - t0)
            return res.results
        except Exception as e:  # transient device errors: retry
            last = e
    raise last


def kernel(SC2_measure, src_keypts, tgt_keypts):
    _launch_wall.clear()
    SC2 = np.ascontiguousarray(SC2_measure[0], dtype=np.float32)      # [512, 2048]
    src = np.ascontiguousarray(src_keypts[0], dtype=np.float32)       # [2048, 3]
    tgt = np.ascontiguousarray(tgt_keypts[0], dtype=np.float32)

    # exact top-200 per seed (desc value, ties -> lower index == lax.top_k)
    knn = np.argsort(-SC2, axis=1, kind='stable')[:, :200]
    knnf = knn.astype(np.float32)                                     # ints < 2048, exact
    ptsflat = np.concatenate([src.T.reshape(3 * NPTS), tgt.T.reshape(3 * NPTS)])
    ptsrows = ptsflat.reshape(SPC, 192)                               # row r: flat[192r:192r+192]

    nc = _get_prog("mega", _prog_mega)
    in_maps = []
    for c in range(NCORES):
        m = np.empty((SPC, 392), np.float32)
        m[:, 0:200] = knnf[c * SPC:(c + 1) * SPC]
        m[:, 200:392] = ptsrows
        in_maps.append({"inp": m})
    for _try in range(3):
        res = _run(nc, in_maps)
        o = np.concatenate([res[c]["out13"] for c in range(NCORES)])  # [512,13]
        cntv = o[:, 0]
        R = o[:, 1:10].reshape(SEEDS, 3, 3)
        t = o[:, 10:13]
        ok = (np.isfinite(cntv).all() and (cntv == np.round(cntv)).all()
              and (cntv >= 0).all() and (cntv <= NPTS).all()
              and np.isfinite(R).all() and np.isfinite(t).all())
        if ok:
            break
    best = int(np.argmax(cntv))
    T = np.zeros((1, 4, 4), np.float32)
    T[0, :3, :3] = R[best]
    T[0, :3, 3] = t[best]
    T[0, 3, 3] = 1.0
    return T


# revision 65
# speedup vs baseline: 1.1178x; 1.1178x over previous
"""Trainium2 Bass kernel for nn_HCF_module (SC2 NMS/registration pipeline).

Single fused device launch (512 seeds sharded 64/core over 8 NeuronCores,
keypoints replicated). Host does only the exact top-200 seed-row selection
(stable argsort = lax.top_k tie order), ships one packed [64, 392] input
per core (200 indices + keypoint scatter rows), and does the final
argmax/T assembly. One input + one output dram tensor per core — per-array
RPC overhead (~40ms/array) dominates the launch wall on axon.

Device program per core (64 seeds on 64 partitions):
  - replicate keypoints to all partitions (scatter rows -> Internal DRAM
    -> stride-0 broadcast DMA), then gather each seed's top-200 coords
    via exact one-hot is_eq over a device-built 0..2047 iota.
  - 4 filter stages k=200/100/50/25: SC2 consistency scores (sqrt-free
    hard-bit test, bit-identical to the validated baseline arithmetic),
    then EXACT top-k/2 selection via unique integer keys 256*sc2 - pos
    (f32-exact integers; DVE max8/match_replace rounds), then one-hot
    is_eq gather of the selected neighbor coords (exact f32 copies).
  - tail: local_sc matrix, 10x power iteration, weighted Kabsch via
    closed-form 3x3 eigensolver + Newton (same op order as the validated
    f32 host model), fitness inlier counts over all 2048 keypoints.
Outputs per seed (packed [64, 13]): inlier count, R (3x3), t (3).
"""
import os as _os
import numpy as np

# Persistent XLA compilation cache: the PJRT wrapper is re-traced per launch
# (fresh closure inside run_bass_kernel_spmd), so without this every warm
# launch re-runs the BIR->NEFF backend pipeline (~0.3s). Must be set before
# jax initializes.
_os.environ.setdefault("JAX_COMPILATION_CACHE_DIR", "/tmp/jax_comp_cache")
_os.environ.setdefault("JAX_PERSISTENT_CACHE_MIN_COMPILE_TIME_SECS", "0")
_os.environ.setdefault("JAX_PERSISTENT_CACHE_MIN_ENTRY_SIZE_BYTES", "0")

F32 = np.float32
T2 = F32(0.1) * F32(0.1)            # 0.010000000707...
TWO_T2 = F32(2.0) * T2
T4 = T2 * T2
INV_T2 = F32(np.float64(1.0) / np.float64(T2))
NCORES = 8
SEEDS = 512
SPC = SEEDS // NCORES               # seeds per core
NPTS = 2048
NEG = -1e30

# filter stages: (k, B, kf, gather-chunk mc)
STAGES = [(200, 20, 100, 50), (100, 20, 50, 50), (50, 25, 25, 25), (25, 25, 12, 12)]

_programs = {}
_launch_wall = []


def _mk_bass(detect_races=True):
    import concourse.bass as bass
    return bass.Bass("TRN2", target_bir_lowering=False,
                     detect_race_conditions=detect_races)


def _prog_mega(debug=False, sync_all=True, trunc=0):
    """Build the fused device program.

    sync_all=True emits a vsem inc+wait after every DVE instruction —
    required by CoreSim's race model (used for validation builds).
    sync_all=False relies on in-order engine execution with the HW's
    per-op pipeline drain, fencing only at ACT/DMA crossings (faster).
    """
    import concourse.mybir as mybir
    from concourse.alu_op_type import AluOpType as OP
    nc = _mk_bass(detect_races=sync_all)
    P = SPC
    # single packed input per core: [0:200) top-200 knn indices (f32 integers)
    # | [200:392) keypts scatter (row r holds pts.flat[r*192:(r+1)*192],
    # pts.flat = src c-major 6144 floats then tgt c-major 6144 floats)
    inp = nc.dram_tensor("inp", [P, 392], mybir.dt.float32, kind="ExternalInput")
    dscr = nc.dram_tensor("dscr", [1, 2 * 3 * NPTS], mybir.dt.float32, kind="Internal")
    # single packed output: col 0 cnt | 1:10 R row-major | 10:13 t
    out13 = nc.dram_tensor("out13", [P, 13], mybir.dt.float32, kind="ExternalOutput")
    dbg_names = []
    if debug:
        dbg_specs = [("dsc1", 200), ("dsc2", 100), ("dsc3", 50), ("dsc4", 25),
                     ("dxf", 36), ("dyf", 36), ("dm", 144),
                     ("dvv", 12), ("dww", 12), ("dh9", 9), ("dk9", 9),
                     ("dlam", 2), ("du1", 3), ("du2", 3), ("dv1", 3)]
        dbg_dram = {n: nc.dram_tensor(n, [P, w], mybir.dt.float32, kind="ExternalOutput")
                    for (n, w) in dbg_specs}
        dbg_names = [n for (n, _) in dbg_specs]

    ctx = nc.ctx
    sb = lambda nm, shape: ctx.enter_context(nc.sbuf_tensor(nm, shape, mybir.dt.float32))[:, :]
    INP = sb("INP", [P, 392])
    IDX = INP[:, 0:200]
    POSI = ctx.enter_context(nc.sbuf_tensor("POSI", [P, 200], mybir.dt.int32))[:, :]
    POS = sb("POS", [P, 200])
    TXa = sb("TXa", [P, 600]); TYa = sb("TYa", [P, 600])
    TXb = sb("TXb", [P, 304]); TYb = sb("TYb", [P, 304])
    TXc = sb("TXc", [P, 304]); TYc = sb("TYc", [P, 304])
    SC2S = sb("SC2S", [P, 200]); H0 = sb("H0", [P, 200])
    KEYP = sb("KEYP", [P, 200]); KEYW = sb("KEYW", [P, 200]); TOPV = sb("TOPV", [P, 104])
    PSRC = sb("PSRC", [P, 3 * NPTS]); PTGT = sb("PTGT", [P, 3 * NPTS])
    VV = sb("VV", [P, 12]); WW = sb("WW", [P, 12])
    OUT13 = sb("OUT13", [P, 13])
    CNTS = OUT13[:, 0:1]; R9S = OUT13[:, 1:10]; T3S = OUT13[:, 10:13]
    FEN = sb("FEN", [P, 1])
    SCR = sb("SCR", [P, 36000])
    IOTA2K = SCR[:, 32768:32768 + NPTS]   # live only during gather_top200
    if debug:
        dbg_sb = {n: sb("sb_" + n, [P, w]) for (n, w) in dbg_specs}

    dins = ctx.enter_context(nc.semaphore())
    dpts = ctx.enter_context(nc.semaphore())
    dout = ctx.enter_context(nc.semaphore())
    vsem = ctx.enter_context(nc.semaphore())
    asem = ctx.enter_context(nc.semaphore())
    gsem = ctx.enter_context(nc.semaphore())

    vcnt = [0]
    acnt = [0]
    sqrt_jobs = []   # (vsem threshold, src AP, dst AP)
    veng = [None]
    marks = {}

    def V(inst):
        if sync_all:
            inst.then_inc(vsem, 1)
            vcnt[0] += 1
            veng[0].wait_ge(vsem, vcnt[0])
        return inst

    def fence():
        # make vsem reflect completion of all vector work so far
        if not sync_all:
            nc.vector.tensor_copy(FEN, FEN).then_inc(vsem, 1)
            vcnt[0] += 1

    def tt(out, a, b, op):
        V(nc.vector.tensor_tensor(out=out, in0=a, in1=b, op=op))

    def ts(out, a, s1, op0, s2=None, op1=None):
        if op1 is None:
            V(nc.vector.tensor_scalar(out, a, s1, None, op0))
        else:
            V(nc.vector.tensor_scalar(out, a, s1, s2, op0, op1))

    def stt(out, in0, s, in1, op0, op1):
        V(nc.vector.scalar_tensor_tensor(out=out, in0=in0, scalar=s, in1=in1,
                                         op0=op0, op1=op1))

    def cp(out, a):
        V(nc.vector.tensor_copy(out, a))

    def red(out, in_, op=None):
        V(nc.vector.tensor_reduce(out=out, in_=in_, axis=mybir.AxisListType.X,
                                  op=op or OP.add))

    def mset(ap, v):
        V(nc.vector.memset(ap, v))

    def rcp(out, in_):
        V(nc.vector.reciprocal(out, in_))

    def act_sqrt(dst, src):
        fence()
        sqrt_jobs.append((vcnt[0], src, dst))
        acnt[0] += 1
        veng[0].wait_ge(asem, acnt[0])

    def sc2_stage(k, B, tx, ty):
        slot = 4000 if k == 200 else 2000
        dxs = SCR[:, 0:B * 3 * k]
        d2a = SCR[:, 12000:12000 + B * k]
        d2b = SCR[:, 12000 + slot:12000 + slot + B * k]
        q = SCR[:, 12000 + 2 * slot:12000 + 2 * slot + B * k]
        pp = SCR[:, 12000 + 3 * slot:12000 + 3 * slot + B * k]
        hard = SCR[:, 12000 + 4 * slot:12000 + 4 * slot + B * k]
        scr2 = SCR[:, 12000 + 5 * slot:12000 + 5 * slot + B * k]
        nb = k // B
        for bi in range(nb):
            a0 = bi * B
            for (src_t, dst) in ((tx, d2a), (ty, d2b)):
                v3 = src_t[:, :3 * k].rearrange("p (c b) -> p c b", c=3)
                rows4 = v3.unsqueeze(1).to_broadcast([P, B, 3, k])
                cols4 = v3[:, :, a0:a0 + B].transpose([0, 2, 1]).unsqueeze(3).to_broadcast([P, B, 3, k])
                dx4 = dxs.rearrange("p (a c b) -> p a c b", a=B, c=3)
                tt(dx4, rows4, cols4, OP.subtract)
                tt(dxs, dxs, dxs, OP.mult)
                d2v = dst.rearrange("p (a b) -> p a b", a=B)
                tt(d2v, dx4[:, :, 0, :], dx4[:, :, 1, :], OP.add)
                tt(d2v, d2v, dx4[:, :, 2, :], OP.add)
            tt(q, d2a, d2b, OP.add)
            tt(pp, d2a, d2b, OP.subtract)
            tt(pp, pp, pp, OP.mult)
            ts(scr2, q, float(TWO_T2), OP.mult, float(T4), OP.subtract)
            tt(hard, pp, scr2, OP.is_lt)
            ts(scr2, q, float(T2), OP.is_lt)
            tt(hard, hard, scr2, OP.max)
            if bi == 0:
                cp(H0[:, :k], hard[:, :k])
            hv = hard.rearrange("p (a b) -> p a b", a=B)
            h0c = H0[:, a0:a0 + B].unsqueeze(2).to_broadcast([P, B, k])
            tt(hv, hv, h0c, OP.mult)
            hT = hv.transpose([0, 2, 1])
            if bi == 0:
                red(SC2S[:, :k], hT)
            else:
                red(scr2[:, :k], hT)
                tt(SC2S[:, :k], SC2S[:, :k], scr2[:, :k], OP.add)

    def key_topk(k, kf):
        # unique integer keys: 256*sc2 - pos; desc key order == (sc2 desc, pos asc)
        ts(KEYP[:, :k], SC2S[:, :k], 256.0, OP.mult)
        tt(KEYP[:, :k], KEYP[:, :k], POS[:, :k], OP.subtract)
        cp(KEYW[:, :k], KEYP[:, :k])
        rounds = (kf + 7) // 8
        for r in range(rounds):
            V(nc.vector.max(out=TOPV[:, r * 8:(r + 1) * 8], in_=KEYW[:, :k]))
            if r < rounds - 1:
                V(nc.vector.match_replace(out=KEYW[:, :k],
                                          in_to_replace=TOPV[:, r * 8:(r + 1) * 8],
                                          in_values=KEYW[:, :k], imm_value=NEG))

    def gather_top200():
        # TXa/TYa[:, c*200+m] = keypts[idx[m], c] via exact one-hot over 2048
        cp(POS, POSI)   # int32 -> f32, exact for 0..199 (iota runs on gpsimd)
        for c in range(10):
            ts(IOTA2K[:, c * 200:(c + 1) * 200], POS, float(200 * c), OP.add)
        ts(IOTA2K[:, 2000:2048], POS[:, 0:48], 2000.0, OP.add)
        mcg = 8
        pv3 = PSRC.rearrange("p (c n) -> p c n", c=3)
        tv3 = PTGT.rearrange("p (c n) -> p c n", c=3)
        for c0 in range(0, 200, mcg):
            w = min(mcg, 200 - c0)
            oh3 = SCR[:, 0:w * NPTS].rearrange("p (m j) -> p m j", m=w)
            tmp3 = SCR[:, mcg * NPTS:mcg * NPTS + w * NPTS].rearrange("p (m j) -> p m j", m=w)
            sel = IDX[:, c0:c0 + w]
            tt(oh3, sel.unsqueeze(2).to_broadcast([P, w, NPTS]),
               IOTA2K.unsqueeze(1).to_broadcast([P, w, NPTS]), OP.is_equal)
            for (src3, t_out) in ((pv3, TXa), (tv3, TYa)):
                for c in range(3):
                    tt(tmp3, oh3,
                       src3[:, c, :].unsqueeze(1).to_broadcast([P, w, NPTS]),
                       OP.mult)
                    red(t_out[:, c * 200 + c0:c * 200 + c0 + w], tmp3)

    def gather(k, kf, mc, tx, ty, ox, oy):
        oh3 = SCR[:, 0:mc * k].rearrange("p (m j) -> p m j", m=mc)
        tmp3 = SCR[:, mc * k:2 * mc * k].rearrange("p (m j) -> p m j", m=mc)
        for c0 in range(0, kf, mc):
            sel = TOPV[:, c0:c0 + mc]
            tt(oh3, sel.unsqueeze(2).to_broadcast([P, mc, k]),
               KEYP[:, :k].unsqueeze(1).to_broadcast([P, mc, k]), OP.is_equal)
            for (t_in, t_out) in ((tx, ox), (ty, oy)):
                for c in range(3):
                    tt(tmp3, oh3,
                       t_in[:, c * k:(c + 1) * k].unsqueeze(1).to_broadcast([P, mc, k]),
                       OP.mult)
                    red(t_out[:, c * kf + c0:c * kf + c0 + mc], tmp3)

    scr_off = [0]

    def alloc(n):
        off = scr_off[0]
        scr_off[0] += n
        assert scr_off[0] <= 12000
        return SCR[:, off:off + n]

    def cross3(out, a, b, tA, tB):
        for i in range(3):
            j, kk = (i + 1) % 3, (i + 2) % 3
            tt(tA, a[:, j:j + 1], b[:, kk:kk + 1], OP.mult)
            tt(tB, a[:, kk:kk + 1], b[:, j:j + 1], OP.mult)
            tt(out[:, i:i + 1], tA, tB, OP.subtract)

    def normalize3(u, nu, ns, rn, t3v, eps=1e-38):
        # u *= 1/sqrt(max(sum(u^2), eps))
        tt(t3v, u, u, OP.mult)
        red(nu, t3v)
        ts(nu, nu, eps, OP.max)
        act_sqrt(ns, nu)
        rcp(rn, ns)
        ts(u, u, rn, OP.mult)

    with nc.Block() as block:
        @block.vector
        def _(vector):
            veng[0] = vector
            mset(FEN, 0.0)
            vector.wait_ge(dins, 16)     # INP DMA
            vector.wait_ge(gsem, 1)      # gpsimd iota
            vector.wait_ge(dpts, 48)     # PSRC/PTGT replicated
            gather_top200()
            curx, cury = TXa, TYa
            for si, (k, B, kf, mc) in enumerate(STAGES):
                nxtx, nxty = (TXb, TYb) if si % 2 == 0 else (TXc, TYc)
                sc2_stage(k, B, curx, cury)
                if trunc == 1 and si == 0:
                    fence()
                    return
                if debug:
                    cp(dbg_sb[["dsc1", "dsc2", "dsc3", "dsc4"][si]], SC2S[:, :k])
                key_topk(k, kf)
                gather(k, kf, mc, curx, cury, nxtx, nxty)
                curx, cury = nxtx, nxty
            if trunc == 2:
                fence()
                return
            # final selected coords: curx[:, :36], cury[:, :36] (c-major, 12 each)
            if debug:
                cp(dbg_sb["dxf"], curx[:, :36])
                cp(dbg_sb["dyf"], cury[:, :36])

            # ---- local_sc matrix M [12x12] ----
            DX = alloc(432)
            A2 = alloc(144); B2 = alloc(144)
            DA = alloc(144); DB = alloc(144)
            CR = alloc(144); M144 = alloc(144); PR = alloc(144)
            for (t_in, d2out) in ((curx, A2), (cury, B2)):
                v3 = t_in[:, :36].rearrange("p (c b) -> p c b", c=3)
                rows4 = v3.unsqueeze(1).to_broadcast([P, 12, 3, 12])
                cols4 = v3.transpose([0, 2, 1]).unsqueeze(3).to_broadcast([P, 12, 3, 12])
                dx4 = DX.rearrange("p (a c b) -> p a c b", a=12, c=3)
                tt(dx4, rows4, cols4, OP.subtract)
                tt(DX, DX, DX, OP.mult)
                d2v = d2out.rearrange("p (a b) -> p a b", a=12)
                tt(d2v, dx4[:, :, 0, :], dx4[:, :, 1, :], OP.add)
                tt(d2v, d2v, dx4[:, :, 2, :], OP.add)
            ts(A2, A2, 1e-12, OP.max)
            ts(B2, B2, 1e-12, OP.max)
            act_sqrt(DA, A2)
            act_sqrt(DB, B2)
            tt(CR, DA, DB, OP.subtract)
            tt(CR, CR, CR, OP.mult)   # |da-db|^2 == (da-db)^2 exactly
            ts(M144, CR, -float(INV_T2), OP.mult, 1.0, OP.add)
            ts(M144, M144, 0.0, OP.max)
            for i in range(12):
                mset(M144[:, 13 * i:13 * i + 1], 0.0)
            if debug:
                cp(dbg_sb["dm"], M144)

            # ---- power iteration (10 iters) ----
            m3 = M144.rearrange("p (i j) -> p i j", i=12)
            VN = alloc(12); T12 = alloc(12)
            N2 = alloc(1); NN = alloc(1); RN = alloc(1)
            mset(VV, 1.0)
            for _ in range(10):
                tt(PR.rearrange("p (i j) -> p i j", i=12), m3,
                   VV.unsqueeze(1).to_broadcast([P, 12, 12]), OP.mult)
                red(VN, PR.rearrange("p (i j) -> p i j", i=12))
                tt(T12, VN, VN, OP.mult)
                red(N2, T12)
                act_sqrt(NN, N2)
                ts(NN, NN, 1e-6, OP.add)
                rcp(RN, NN)
                ts(VV, VN, RN, OP.mult)
            if debug:
                cp(dbg_sb["dvv"], VV)
            # w = v / (sum(v) + 1e-6)
            S1 = alloc(1); RS = alloc(1)
            red(S1, VV)
            ts(S1, S1, 1e-6, OP.add)
            rcp(RS, S1)
            ts(WW, VV, RS, OP.mult)
            if debug:
                cp(dbg_sb["dww"], WW)

            # ---- weighted Kabsch ----
            a3 = curx[:, :36].rearrange("p (c b) -> p c b", c=3)
            b3 = cury[:, :36].rearrange("p (c b) -> p c b", c=3)
            WS = alloc(1); RWS = alloc(1)
            red(WS, WW)
            ts(WS, WS, 1e-6, OP.add)
            rcp(RWS, WS)
            WA = alloc(36); SA = alloc(3); CA = alloc(3); CB = alloc(3)
            AM = alloc(36); BM = alloc(36); WAM = alloc(36)
            wb = WW.unsqueeze(1).to_broadcast([P, 3, 12])
            tt(WA.rearrange("p (c b) -> p c b", c=3), a3, wb, OP.mult)
            red(SA, WA.rearrange("p (c b) -> p c b", c=3))
            ts(CA, SA, RWS, OP.mult)
            tt(WA.rearrange("p (c b) -> p c b", c=3), b3, wb, OP.mult)
            red(SA, WA.rearrange("p (c b) -> p c b", c=3))
            ts(CB, SA, RWS, OP.mult)
            tt(AM.rearrange("p (c b) -> p c b", c=3), a3,
               CA.unsqueeze(2).to_broadcast([P, 3, 12]), OP.subtract)
            tt(BM.rearrange("p (c b) -> p c b", c=3), b3,
               CB.unsqueeze(2).to_broadcast([P, 3, 12]), OP.subtract)
            tt(WAM.rearrange("p (c b) -> p c b", c=3),
               AM.rearrange("p (c b) -> p c b", c=3), wb, OP.mult)
            HP = alloc(108); H9 = alloc(9)
            tt(HP.rearrange("p (i j b) -> p i j b", i=3, j=3),
               WAM.rearrange("p (c b) -> p c b", c=3).unsqueeze(2).to_broadcast([P, 3, 3, 12]),
               BM.rearrange("p (c b) -> p c b", c=3).unsqueeze(1).to_broadcast([P, 3, 3, 12]),
               OP.mult)
            red(H9, HP.rearrange("p (i j b) -> p i j b", i=3, j=3))
            if debug:
                cp(dbg_sb["dh9"], H9)
            KP = alloc(27); K9 = alloc(9)
            h3v = H9.rearrange("p (i j) -> p i j", i=3)
            tt(KP.rearrange("p (i l j) -> p i l j", i=3, l=3),
               h3v.unsqueeze(2).to_broadcast([P, 3, 3, 3]),
               h3v.unsqueeze(1).to_broadcast([P, 3, 3, 3]), OP.mult)
            red(K9, KP.rearrange("p (i l j) -> p i l j", i=3, l=3))
            if debug:
                cp(dbg_sb["dk9"], K9)

            # ---- closed-form eigenvalues of K (3x3 sym PSD) ----
            c1_ = lambda i: K9[:, i:i + 1]
            QQ = alloc(1)
            tt(QQ, c1_(0), c1_(4), OP.add)
            tt(QQ, QQ, c1_(8), OP.add)
            ts(QQ, QQ, float(F32(1.0 / 3.0)), OP.mult)
            KD = alloc(3)   # K00-qq, K11-qq, K22-qq
            for di, src_i in enumerate((0, 4, 8)):
                tt(KD[:, di:di + 1], c1_(src_i), QQ, OP.subtract)
            P1 = alloc(1); TTa = alloc(1); TTb = alloc(1)
            tt(P1, c1_(1), c1_(1), OP.mult)
            tt(TTa, c1_(2), c1_(2), OP.mult)
            tt(P1, P1, TTa, OP.add)
            tt(TTa, c1_(5), c1_(5), OP.mult)
            tt(P1, P1, TTa, OP.add)
            P2 = alloc(1)
            tt(P2, KD[:, 0:1], KD[:, 0:1], OP.mult)
            tt(TTa, KD[:, 1:2], KD[:, 1:2], OP.mult)
            tt(P2, P2, TTa, OP.add)
            tt(TTa, KD[:, 2:3], KD[:, 2:3], OP.mult)
            tt(P2, P2, TTa, OP.add)
            ts(TTa, P1, 2.0, OP.mult)
            tt(P2, P2, TTa, OP.add)
            PV = alloc(1); RP = alloc(1)
            ts(PV, P2, float(F32(1.0 / 6.0)), OP.mult)
            act_sqrt(PV, PV)
            ts(TTa, PV, 1e-30, OP.max)
            rcp(RP, TTa)
            BV = alloc(6)   # B00,B11,B22,B01,B02,B12
            for bi_, src in enumerate((KD[:, 0:1], KD[:, 1:2], KD[:, 2:3],
                                       c1_(1), c1_(2), c1_(5))):
                ts(BV[:, bi_:bi_ + 1], src, RP, OP.mult)
            B00, B11, B22 = BV[:, 0:1], BV[:, 1:2], BV[:, 2:3]
            B01, B02, B12 = BV[:, 3:4], BV[:, 4:5], BV[:, 5:6]
            DET = alloc(1); TTc = alloc(1)
            # t1 = B00*(B11*B22 - B12*B12)
            tt(TTa, B11, B22, OP.mult)
            tt(TTb, B12, B12, OP.mult)
            tt(TTa, TTa, TTb, OP.subtract)
            tt(DET, B00, TTa, OP.mult)
            # t2 = B01*(B01*B22 - B12*B02); det = t1 - t2
            tt(TTa, B01, B22, OP.mult)
            tt(TTb, B12, B02, OP.mult)
            tt(TTa, TTa, TTb, OP.subtract)
            tt(TTc, B01, TTa, OP.mult)
            tt(DET, DET, TTc, OP.subtract)
            # t3 = B02*(B01*B12 - B11*B02); det = det + t3
            tt(TTa, B01, B12, OP.mult)
            tt(TTb, B11, B02, OP.mult)
            tt(TTa, TTa, TTb, OP.subtract)
            tt(TTc, B02, TTa, OP.mult)
            tt(DET, DET, TTc, OP.add)
            RV = alloc(1)
            ts(RV, DET, 0.5, OP.mult)
            ts(RV, RV, -1.0, OP.max)
            ts(RV, RV, 1.0, OP.min)
            CC = alloc(1); C2 = alloc(1); C3 = alloc(1)
            FF = alloc(1); FP = alloc(1); RFP = alloc(1)
            mset(CC, 1.0)
            for _ in range(6):
                tt(C2, CC, CC, OP.mult)
                tt(C3, C2, CC, OP.mult)
                ts(FF, C3, 4.0, OP.mult)
                ts(TTa, CC, 3.0, OP.mult)
                tt(FF, FF, TTa, OP.subtract)
                tt(FF, FF, RV, OP.subtract)
                ts(FP, C2, 12.0, OP.mult, 3.0, OP.subtract)
                ts(FP, FP, 1e-6, OP.max)
                rcp(RFP, FP)
                tt(TTa, FF, RFP, OP.mult)
                tt(CC, CC, TTa, OP.subtract)
                ts(CC, CC, 0.5, OP.max)
                ts(CC, CC, 1.0, OP.min)
            SS = alloc(1)
            tt(SS, CC, CC, OP.mult)
            ts(SS, SS, -1.0, OP.mult, 1.0, OP.add)
            ts(SS, SS, 0.0, OP.max)
            act_sqrt(SS, SS)
            LAM1 = alloc(1); LAM2 = alloc(1)
            ts(TTa, PV, 2.0, OP.mult)
            tt(TTa, TTa, CC, OP.mult)
            tt(LAM1, QQ, TTa, OP.add)
            ts(TTa, CC, -0.5, OP.mult)
            ts(TTb, SS, float(F32(np.sqrt(3.0) / 2.0)), OP.mult)
            tt(TTa, TTa, TTb, OP.add)
            ts(TTb, PV, 2.0, OP.mult)
            tt(TTa, TTa, TTb, OP.mult)
            tt(LAM2, QQ, TTa, OP.add)
            if debug:
                cp(dbg_sb["dlam"][:, 0:1], LAM1)
                cp(dbg_sb["dlam"][:, 1:2], LAM2)

            # ---- eigenvectors ----
            AK = alloc(9)
            C1v = alloc(3); C2v = alloc(3); C3v = alloc(3)
            N1 = alloc(1); N2e = alloc(1); N3e = alloc(1)
            MA = alloc(1); MB = alloc(1); MC = alloc(1)
            T3v = alloc(3); NU = alloc(1); NS = alloc(1); RNU = alloc(1)
            U1 = alloc(3); U2 = alloc(3); U3 = alloc(3)

            def eigvec(lam, uout):
                cp(AK, K9)
                for d in range(3):
                    tt(AK[:, 4 * d:4 * d + 1], AK[:, 4 * d:4 * d + 1], lam, OP.subtract)
                r0, r1, r2 = AK[:, 0:3], AK[:, 3:6], AK[:, 6:9]
                cross3(C1v, r0, r1, TTa, TTb)
                cross3(C2v, r1, r2, TTa, TTb)
                cross3(C3v, r2, r0, TTa, TTb)
                for (cv, nv) in ((C1v, N1), (C2v, N2e), (C3v, N3e)):
                    tt(T3v, cv, cv, OP.mult)
                    red(nv, T3v)
                tt(MA, N1, N2e, OP.is_ge)
                tt(TTa, N1, N3e, OP.is_ge)
                tt(MA, MA, TTa, OP.mult)
                ts(TTa, MA, -1.0, OP.mult, 1.0, OP.add)     # 1 - a1
                tt(MB, N2e, N3e, OP.is_ge)
                tt(MB, TTa, MB, OP.mult)                     # a2
                tt(MC, TTa, MB, OP.subtract)                 # a3
                ts(uout, C1v, MA, OP.mult)
                ts(T3v, C2v, MB, OP.mult)
                tt(uout, uout, T3v, OP.add)
                ts(T3v, C3v, MC, OP.mult)
                tt(uout, uout, T3v, OP.add)
                normalize3(uout, NU, NS, RNU, T3v)

            eigvec(LAM1, U1)
            eigvec(LAM2, U2)
            if debug:
                cp(dbg_sb["du1"], U1)
            # Gram-Schmidt u2 against u1
            DOT = alloc(1)
            tt(T3v, U1, U2, OP.mult)
            red(DOT, T3v)
            ts(T3v, U1, DOT, OP.mult)
            tt(U2, U2, T3v, OP.subtract)
            normalize3(U2, NU, NS, RNU, T3v)
            if debug:
                cp(dbg_sb["du2"], U2)
            cross3(U3, U1, U2, TTa, TTb)

            # v_i = normalize(H^T u_i); v3 = v1 x v2
            HP2 = alloc(9)
            V1 = alloc(3); V2 = alloc(3); V3 = alloc(3)
            ht3 = H9.rearrange("p (i j) -> p i j", i=3).transpose([0, 2, 1])
            for (uin, vout) in ((U1, V1), (U2, V2)):
                tt(HP2.rearrange("p (i j) -> p i j", i=3), ht3,
                   uin.unsqueeze(1).to_broadcast([P, 3, 3]), OP.mult)
                red(vout, HP2.rearrange("p (i j) -> p i j", i=3))
                normalize3(vout, NU, NS, RNU, T3v)
            if debug:
                cp(dbg_sb["dv1"], V1)
            cross3(V3, V1, V2, TTa, TTb)

            # R = v1 u1^T + v2 u2^T + v3 u3^T ;  t = cB - R cA
            OP9 = alloc(9)
            tt(R9S.rearrange("p (i j) -> p i j", i=3),
               V1.unsqueeze(2).to_broadcast([P, 3, 3]),
               U1.unsqueeze(1).to_broadcast([P, 3, 3]), OP.mult)
            for (vv_, uu_) in ((V2, U2), (V3, U3)):
                tt(OP9.rearrange("p (i j) -> p i j", i=3),
                   vv_.unsqueeze(2).to_broadcast([P, 3, 3]),
                   uu_.unsqueeze(1).to_broadcast([P, 3, 3]), OP.mult)
                tt(R9S, R9S, OP9, OP.add)
            tt(OP9.rearrange("p (i j) -> p i j", i=3),
               R9S.rearrange("p (i j) -> p i j", i=3),
               CA.unsqueeze(1).to_broadcast([P, 3, 3]), OP.mult)
            RC = alloc(3)
            red(RC, OP9.rearrange("p (i j) -> p i j", i=3))
            tt(T3S, CB, RC, OP.subtract)

            if trunc == 3:
                fence()
                return
            # ---- fitness: count ||R x + t - y|| < 0.1 over all 2048 pts ----
            DC = SCR[:, 0:6144].rearrange("p (c n) -> p c n", c=3)
            ACC = SCR[:, 6144:6144 + 2048]
            L2S = SCR[:, 8192:8192 + 2048]
            SQ = SCR[:, 10240:10240 + 2048]
            xv = PSRC.rearrange("p (c n) -> p c n", c=3)
            yv = PTGT.rearrange("p (c n) -> p c n", c=3)
            for c in range(3):
                ts(ACC, xv[:, 0, :], R9S[:, 3 * c:3 * c + 1], OP.mult,
                   T3S[:, c:c + 1], OP.add)
                stt(ACC, xv[:, 1, :], R9S[:, 3 * c + 1:3 * c + 2], ACC, OP.mult, OP.add)
                stt(ACC, xv[:, 2, :], R9S[:, 3 * c + 2:3 * c + 3], ACC, OP.mult, OP.add)
                tt(DC[:, c, :], ACC, yv[:, c, :], OP.subtract)
            tt(L2S, DC[:, 0, :], DC[:, 0, :], OP.mult)
            tt(SQ, DC[:, 1, :], DC[:, 1, :], OP.mult)
            tt(L2S, L2S, SQ, OP.add)
            tt(SQ, DC[:, 2, :], DC[:, 2, :], OP.mult)
            tt(L2S, L2S, SQ, OP.add)
            ts(SQ, L2S, float(T2), OP.is_lt)
            red(CNTS, SQ)
            fence()

        @block.scalar
        def _(scalar):
            from concourse import mybir as mb
            for (vt, src, dst) in sqrt_jobs:
                scalar.wait_ge(vsem, vt)
                nc.scalar.sqrt(dst, src).then_inc(asem, 1)

        @block.gpsimd
        def _(gpsimd):
            gpsimd.dma_start(INP, inp[:, :]).then_inc(dins, 16)
            gpsimd.iota(POSI, pattern=[[1, 200]], base=0,
                        channel_multiplier=0).then_inc(gsem, 1)
            # rebuild replicated keypoint rows: scatter -> DRAM -> broadcast
            gpsimd.wait_ge(dins, 16)
            gpsimd.dma_start(dscr[0:1, :].rearrange("p (a b) -> p a b", a=P),
                             INP[:, 200:392]).then_inc(dpts, 16)
            gpsimd.wait_ge(dpts, 16)
            gpsimd.dma_start(PSRC, dscr[0:1, 0:3 * NPTS].to_broadcast([P, 3 * NPTS])).then_inc(dpts, 16)
            gpsimd.dma_start(PTGT, dscr[0:1, 3 * NPTS:6 * NPTS].to_broadcast([P, 3 * NPTS])).then_inc(dpts, 16)
            gpsimd.wait_ge(vsem, vcnt[0])
            nout = 1 + len(dbg_names)
            gpsimd.dma_start(out13[:, :], OUT13).then_inc(dout, 16)
            if debug:
                for n_ in dbg_names:
                    gpsimd.dma_start(dbg_dram[n_][:, :], dbg_sb[n_]).then_inc(dout, 16)
            gpsimd.wait_ge(dout, 16 * nout)
    return nc


def _get_prog(key, builder):
    if key not in _programs:
        _programs[key] = builder()
    return _programs[key]


_cache_cfg = []


def _enable_jax_cache():
    if _cache_cfg:
        return
    _cache_cfg.append(1)
    try:
        import jax
        jax.config.update("jax_compilation_cache_dir", "/tmp/jax_comp_cache")
        jax.config.update("jax_persistent_cache_min_compile_time_secs", 0)
        jax.config.update("jax_persistent_cache_min_entry_size_bytes", 0)
    except Exception:
        pass


def _run(nc, in_maps):
    import time
    _enable_jax_cache()
    from concourse.bass_utils import run_bass_kernel_spmd
    last = None
    for attempt in range(3):
        try:
            t0 = time.time()
            res = run_bass_kernel_spmd(nc, in_maps, core_ids=list(range(NCORES)))
            _launch_wall.append(time.time() - t0)
            return res.results
        except Exception as e:  # transient device errors: retry
            last = e
    raise last


def kernel(SC2_measure, src_keypts, tgt_keypts):
    _launch_wall.clear()
    SC2 = np.ascontiguousarray(SC2_measure[0], dtype=np.float32)      # [512, 2048]
    src = np.ascontiguousarray(src_keypts[0], dtype=np.float32)       # [2048, 3]
    tgt = np.ascontiguousarray(tgt_keypts[0], dtype=np.float32)

    # exact top-200 per seed (desc value, ties -> lower index == lax.top_k)
    knn = np.argsort(-SC2, axis=1, kind='stable')[:, :200]
    knnf = knn.astype(np.float32)                                     # ints < 2048, exact
    ptsflat = np.concatenate([src.T.reshape(3 * NPTS), tgt.T.reshape(3 * NPTS)])
    ptsrows = ptsflat.reshape(SPC, 192)                               # row r: flat[192r:192r+192]

    nc = _get_prog("mega", _prog_mega)
    in_maps = []
    for c in range(NCORES):
        m = np.empty((SPC, 392), np.float32)
        m[:, 0:200] = knnf[c * SPC:(c + 1) * SPC]
        m[:, 200:392] = ptsrows
        in_maps.append({"inp": m})
    for _try in range(3):
        res = _run(nc, in_maps)
        o = np.concatenate([res[c]["out13"] for c in range(NCORES)])  # [512,13]
        cntv = o[:, 0]
        R = o[:, 1:10].reshape(SEEDS, 3, 3)
        t = o[:, 10:13]
        ok = (np.isfinite(cntv).all() and (cntv == np.round(cntv)).all()
              and (cntv >= 0).all() and (cntv <= NPTS).all()
              and np.isfinite(R).all() and np.isfinite(t).all())
        if ok:
            break
    best = int(np.argmax(cntv))
    T = np.zeros((1, 4, 4), np.float32)
    T[0, :3, :3] = R[best]
    T[0, :3, 3] = t[best]
    T[0, 3, 3] = 1.0
    return T


# revision 66
# speedup vs baseline: 1.3247x; 1.1851x over previous
"""Trainium2 Bass kernel for nn_HCF_module (SC2 NMS/registration pipeline).

Single fused device launch (512 seeds sharded 64/core over 8 NeuronCores,
keypoints replicated). Host does only the exact top-200 seed-row selection
(stable argsort = lax.top_k tie order), ships one packed [64, 392] input
per core (200 indices + keypoint scatter rows), and does the final
argmax/T assembly. One input + one output dram tensor per core — per-array
RPC overhead (~40ms/array) dominates the launch wall on axon.

Device program per core (64 seeds on 64 partitions):
  - replicate keypoints to all partitions (scatter rows -> Internal DRAM
    -> stride-0 broadcast DMA), then gather each seed's top-200 coords
    via exact one-hot is_eq over a device-built 0..2047 iota.
  - 4 filter stages k=200/100/50/25: SC2 consistency scores (sqrt-free
    hard-bit test, bit-identical to the validated baseline arithmetic),
    then EXACT top-k/2 selection via unique integer keys 256*sc2 - pos
    (f32-exact integers; DVE max8/match_replace rounds), then one-hot
    is_eq gather of the selected neighbor coords (exact f32 copies).
  - tail: local_sc matrix, 10x power iteration, weighted Kabsch via
    closed-form 3x3 eigensolver + Newton (same op order as the validated
    f32 host model), fitness inlier counts over all 2048 keypoints.
Outputs per seed (packed [64, 13]): inlier count, R (3x3), t (3).
"""
import os as _os
import numpy as np

# Persistent XLA compilation cache: the PJRT wrapper is re-traced per launch
# (fresh closure inside run_bass_kernel_spmd), so without this every warm
# launch re-runs the BIR->NEFF backend pipeline (~0.3s). Must be set before
# jax initializes.
_os.environ.setdefault("JAX_COMPILATION_CACHE_DIR", "/tmp/jax_comp_cache")
_os.environ.setdefault("JAX_PERSISTENT_CACHE_MIN_COMPILE_TIME_SECS", "0")
_os.environ.setdefault("JAX_PERSISTENT_CACHE_MIN_ENTRY_SIZE_BYTES", "0")

F32 = np.float32
T2 = F32(0.1) * F32(0.1)            # 0.010000000707...
TWO_T2 = F32(2.0) * T2
T4 = T2 * T2
INV_T2 = F32(np.float64(1.0) / np.float64(T2))
NCORES = 8
SEEDS = 512
SPC = SEEDS // NCORES               # seeds per core
NPTS = 2048
NEG = -1e30

# filter stages: (k, B, kf, gather-chunk mc)
STAGES = [(200, 20, 100, 50), (100, 20, 50, 50), (50, 25, 25, 25), (25, 25, 12, 12)]

_programs = {}
_launch_wall = []


def _mk_bass(detect_races=True):
    import concourse.bass as bass
    return bass.Bass("TRN2", target_bir_lowering=False,
                     detect_race_conditions=detect_races)


def _prog_mega(debug=False, sync_all=True, trunc=0):
    """Build the fused device program.

    sync_all=True emits a vsem inc+wait after every DVE instruction —
    required by CoreSim's race model (used for validation builds).
    sync_all=False relies on in-order engine execution with the HW's
    per-op pipeline drain, fencing only at ACT/DMA crossings (faster).
    """
    import concourse.mybir as mybir
    from concourse.alu_op_type import AluOpType as OP
    nc = _mk_bass(detect_races=sync_all)
    P = SPC
    # single packed input per core: [0:200) top-200 knn indices (f32 integers)
    # | [200:392) keypts scatter (row r holds pts.flat[r*192:(r+1)*192],
    # pts.flat = src c-major 6144 floats then tgt c-major 6144 floats)
    inp = nc.dram_tensor("inp", [P, 392], mybir.dt.float32, kind="ExternalInput")
    dscr = nc.dram_tensor("dscr", [1, 2 * 3 * NPTS], mybir.dt.float32, kind="Internal")
    # single packed output: col 0 cnt | 1:10 R row-major | 10:13 t
    out13 = nc.dram_tensor("out13", [P, 13], mybir.dt.float32, kind="ExternalOutput")
    dbg_names = []
    if debug:
        dbg_specs = [("dsc1", 200), ("dsc2", 100), ("dsc3", 50), ("dsc4", 25),
                     ("dxf", 36), ("dyf", 36), ("dm", 144),
                     ("dvv", 12), ("dww", 12), ("dh9", 9), ("dk9", 9),
                     ("dlam", 2), ("du1", 3), ("du2", 3), ("dv1", 3)]
        dbg_dram = {n: nc.dram_tensor(n, [P, w], mybir.dt.float32, kind="ExternalOutput")
                    for (n, w) in dbg_specs}
        dbg_names = [n for (n, _) in dbg_specs]

    ctx = nc.ctx
    sb = lambda nm, shape: ctx.enter_context(nc.sbuf_tensor(nm, shape, mybir.dt.float32))[:, :]
    INP = sb("INP", [P, 392])
    IDX = INP[:, 0:200]
    POSI = ctx.enter_context(nc.sbuf_tensor("POSI", [P, 200], mybir.dt.int32))[:, :]
    POS = sb("POS", [P, 200])
    TXa = sb("TXa", [P, 600]); TYa = sb("TYa", [P, 600])
    TXb = sb("TXb", [P, 304]); TYb = sb("TYb", [P, 304])
    TXc = sb("TXc", [P, 304]); TYc = sb("TYc", [P, 304])
    SC2S = sb("SC2S", [P, 200]); H0 = sb("H0", [P, 200])
    KEYP = sb("KEYP", [P, 200]); KEYW = sb("KEYW", [P, 200]); TOPV = sb("TOPV", [P, 104])
    PSRC = sb("PSRC", [P, 3 * NPTS]); PTGT = sb("PTGT", [P, 3 * NPTS])
    VV = sb("VV", [P, 12]); WW = sb("WW", [P, 12])
    OUT13 = sb("OUT13", [P, 13])
    CNTS = OUT13[:, 0:1]; R9S = OUT13[:, 1:10]; T3S = OUT13[:, 10:13]
    FEN = sb("FEN", [P, 1])
    SCR = sb("SCR", [P, 36000])
    IOTA2K = SCR[:, 32768:32768 + NPTS]   # live only during gather_top200
    if debug:
        dbg_sb = {n: sb("sb_" + n, [P, w]) for (n, w) in dbg_specs}

    dins = ctx.enter_context(nc.semaphore())
    dpts = ctx.enter_context(nc.semaphore())
    dout = ctx.enter_context(nc.semaphore())
    vsem = ctx.enter_context(nc.semaphore())
    asem = ctx.enter_context(nc.semaphore())
    gsem = ctx.enter_context(nc.semaphore())

    vcnt = [0]
    acnt = [0]
    sqrt_jobs = []   # (vsem threshold, src AP, dst AP)
    veng = [None]
    marks = {}

    def V(inst):
        # embed the order-edge in the instruction itself: wait for the
        # previous instruction's vsem value, inc after completion. Same
        # fence semantics as a standalone wait, half the BIR entries.
        if sync_all:
            if vcnt[0] > 0:
                inst.wait_op(vsem, vcnt[0], "sem-ge")
            inst.then_inc(vsem, 1)
            vcnt[0] += 1
        return inst

    def fence():
        # make vsem reflect completion of all vector work so far
        if not sync_all:
            nc.vector.tensor_copy(FEN, FEN).then_inc(vsem, 1)
            vcnt[0] += 1

    def tt(out, a, b, op):
        V(nc.vector.tensor_tensor(out=out, in0=a, in1=b, op=op))

    def ts(out, a, s1, op0, s2=None, op1=None):
        if op1 is None:
            V(nc.vector.tensor_scalar(out, a, s1, None, op0))
        else:
            V(nc.vector.tensor_scalar(out, a, s1, s2, op0, op1))

    def stt(out, in0, s, in1, op0, op1):
        V(nc.vector.scalar_tensor_tensor(out=out, in0=in0, scalar=s, in1=in1,
                                         op0=op0, op1=op1))

    def cp(out, a):
        V(nc.vector.tensor_copy(out, a))

    def red(out, in_, op=None):
        V(nc.vector.tensor_reduce(out=out, in_=in_, axis=mybir.AxisListType.X,
                                  op=op or OP.add))

    def mset(ap, v):
        V(nc.vector.memset(ap, v))

    def rcp(out, in_):
        V(nc.vector.reciprocal(out, in_))

    def act_sqrt(dst, src):
        fence()
        sqrt_jobs.append((vcnt[0], src, dst))
        acnt[0] += 1
        veng[0].wait_ge(asem, acnt[0])

    def sc2_stage(k, B, tx, ty):
        slot = 4000 if k == 200 else 2000
        dxs = SCR[:, 0:B * 3 * k]
        d2a = SCR[:, 12000:12000 + B * k]
        d2b = SCR[:, 12000 + slot:12000 + slot + B * k]
        q = SCR[:, 12000 + 2 * slot:12000 + 2 * slot + B * k]
        pp = SCR[:, 12000 + 3 * slot:12000 + 3 * slot + B * k]
        hard = SCR[:, 12000 + 4 * slot:12000 + 4 * slot + B * k]
        scr2 = SCR[:, 12000 + 5 * slot:12000 + 5 * slot + B * k]
        nb = k // B
        for bi in range(nb):
            a0 = bi * B
            for (src_t, dst) in ((tx, d2a), (ty, d2b)):
                v3 = src_t[:, :3 * k].rearrange("p (c b) -> p c b", c=3)
                rows4 = v3.unsqueeze(1).to_broadcast([P, B, 3, k])
                cols4 = v3[:, :, a0:a0 + B].transpose([0, 2, 1]).unsqueeze(3).to_broadcast([P, B, 3, k])
                dx4 = dxs.rearrange("p (a c b) -> p a c b", a=B, c=3)
                tt(dx4, rows4, cols4, OP.subtract)
                tt(dxs, dxs, dxs, OP.mult)
                d2v = dst.rearrange("p (a b) -> p a b", a=B)
                tt(d2v, dx4[:, :, 0, :], dx4[:, :, 1, :], OP.add)
                tt(d2v, d2v, dx4[:, :, 2, :], OP.add)
            tt(q, d2a, d2b, OP.add)
            tt(pp, d2a, d2b, OP.subtract)
            tt(pp, pp, pp, OP.mult)
            ts(scr2, q, float(TWO_T2), OP.mult, float(T4), OP.subtract)
            tt(hard, pp, scr2, OP.is_lt)
            ts(scr2, q, float(T2), OP.is_lt)
            tt(hard, hard, scr2, OP.max)
            if bi == 0:
                cp(H0[:, :k], hard[:, :k])
            hv = hard.rearrange("p (a b) -> p a b", a=B)
            h0c = H0[:, a0:a0 + B].unsqueeze(2).to_broadcast([P, B, k])
            tt(hv, hv, h0c, OP.mult)
            hT = hv.transpose([0, 2, 1])
            if bi == 0:
                red(SC2S[:, :k], hT)
            else:
                red(scr2[:, :k], hT)
                tt(SC2S[:, :k], SC2S[:, :k], scr2[:, :k], OP.add)

    def key_topk(k, kf):
        # unique integer keys: 256*sc2 - pos; desc key order == (sc2 desc, pos asc)
        ts(KEYP[:, :k], SC2S[:, :k], 256.0, OP.mult)
        tt(KEYP[:, :k], KEYP[:, :k], POS[:, :k], OP.subtract)
        cp(KEYW[:, :k], KEYP[:, :k])
        rounds = (kf + 7) // 8
        for r in range(rounds):
            V(nc.vector.max(out=TOPV[:, r * 8:(r + 1) * 8], in_=KEYW[:, :k]))
            if r < rounds - 1:
                V(nc.vector.match_replace(out=KEYW[:, :k],
                                          in_to_replace=TOPV[:, r * 8:(r + 1) * 8],
                                          in_values=KEYW[:, :k], imm_value=NEG))

    def gather_top200():
        # TXa/TYa[:, c*200+m] = keypts[idx[m], c] via exact one-hot over 2048
        cp(POS, POSI)   # int32 -> f32, exact for 0..199 (iota runs on gpsimd)
        for c in range(10):
            ts(IOTA2K[:, c * 200:(c + 1) * 200], POS, float(200 * c), OP.add)
        ts(IOTA2K[:, 2000:2048], POS[:, 0:48], 2000.0, OP.add)
        mcg = 8
        pv3 = PSRC.rearrange("p (c n) -> p c n", c=3)
        tv3 = PTGT.rearrange("p (c n) -> p c n", c=3)
        for c0 in range(0, 200, mcg):
            w = min(mcg, 200 - c0)
            oh3 = SCR[:, 0:w * NPTS].rearrange("p (m j) -> p m j", m=w)
            tmp3 = SCR[:, mcg * NPTS:mcg * NPTS + w * NPTS].rearrange("p (m j) -> p m j", m=w)
            sel = IDX[:, c0:c0 + w]
            tt(oh3, sel.unsqueeze(2).to_broadcast([P, w, NPTS]),
               IOTA2K.unsqueeze(1).to_broadcast([P, w, NPTS]), OP.is_equal)
            for (src3, t_out) in ((pv3, TXa), (tv3, TYa)):
                for c in range(3):
                    tt(tmp3, oh3,
                       src3[:, c, :].unsqueeze(1).to_broadcast([P, w, NPTS]),
                       OP.mult)
                    red(t_out[:, c * 200 + c0:c * 200 + c0 + w], tmp3)

    def gather(k, kf, mc, tx, ty, ox, oy):
        oh3 = SCR[:, 0:mc * k].rearrange("p (m j) -> p m j", m=mc)
        tmp3 = SCR[:, mc * k:2 * mc * k].rearrange("p (m j) -> p m j", m=mc)
        for c0 in range(0, kf, mc):
            sel = TOPV[:, c0:c0 + mc]
            tt(oh3, sel.unsqueeze(2).to_broadcast([P, mc, k]),
               KEYP[:, :k].unsqueeze(1).to_broadcast([P, mc, k]), OP.is_equal)
            for (t_in, t_out) in ((tx, ox), (ty, oy)):
                for c in range(3):
                    tt(tmp3, oh3,
                       t_in[:, c * k:(c + 1) * k].unsqueeze(1).to_broadcast([P, mc, k]),
                       OP.mult)
                    red(t_out[:, c * kf + c0:c * kf + c0 + mc], tmp3)

    scr_off = [0]

    def alloc(n):
        off = scr_off[0]
        scr_off[0] += n
        assert scr_off[0] <= 12000
        return SCR[:, off:off + n]

    def cross3(out, a, b, tA, tB):
        for i in range(3):
            j, kk = (i + 1) % 3, (i + 2) % 3
            tt(tA, a[:, j:j + 1], b[:, kk:kk + 1], OP.mult)
            tt(tB, a[:, kk:kk + 1], b[:, j:j + 1], OP.mult)
            tt(out[:, i:i + 1], tA, tB, OP.subtract)

    def normalize3(u, nu, ns, rn, t3v, eps=1e-38):
        # u *= 1/sqrt(max(sum(u^2), eps))
        tt(t3v, u, u, OP.mult)
        red(nu, t3v)
        ts(nu, nu, eps, OP.max)
        act_sqrt(ns, nu)
        rcp(rn, ns)
        ts(u, u, rn, OP.mult)

    with nc.Block() as block:
        @block.vector
        def _(vector):
            veng[0] = vector
            mset(FEN, 0.0)
            vector.wait_ge(dins, 16)     # INP DMA
            vector.wait_ge(gsem, 1)      # gpsimd iota
            vector.wait_ge(dpts, 48)     # PSRC/PTGT replicated
            gather_top200()
            curx, cury = TXa, TYa
            for si, (k, B, kf, mc) in enumerate(STAGES):
                nxtx, nxty = (TXb, TYb) if si % 2 == 0 else (TXc, TYc)
                sc2_stage(k, B, curx, cury)
                if trunc == 1 and si == 0:
                    fence()
                    return
                if debug:
                    cp(dbg_sb[["dsc1", "dsc2", "dsc3", "dsc4"][si]], SC2S[:, :k])
                key_topk(k, kf)
                gather(k, kf, mc, curx, cury, nxtx, nxty)
                curx, cury = nxtx, nxty
            if trunc == 2:
                fence()
                return
            # final selected coords: curx[:, :36], cury[:, :36] (c-major, 12 each)
            if debug:
                cp(dbg_sb["dxf"], curx[:, :36])
                cp(dbg_sb["dyf"], cury[:, :36])

            # ---- local_sc matrix M [12x12] ----
            DX = alloc(432)
            A2 = alloc(144); B2 = alloc(144)
            DA = alloc(144); DB = alloc(144)
            CR = alloc(144); M144 = alloc(144); PR = alloc(144)
            for (t_in, d2out) in ((curx, A2), (cury, B2)):
                v3 = t_in[:, :36].rearrange("p (c b) -> p c b", c=3)
                rows4 = v3.unsqueeze(1).to_broadcast([P, 12, 3, 12])
                cols4 = v3.transpose([0, 2, 1]).unsqueeze(3).to_broadcast([P, 12, 3, 12])
                dx4 = DX.rearrange("p (a c b) -> p a c b", a=12, c=3)
                tt(dx4, rows4, cols4, OP.subtract)
                tt(DX, DX, DX, OP.mult)
                d2v = d2out.rearrange("p (a b) -> p a b", a=12)
                tt(d2v, dx4[:, :, 0, :], dx4[:, :, 1, :], OP.add)
                tt(d2v, d2v, dx4[:, :, 2, :], OP.add)
            ts(A2, A2, 1e-12, OP.max)
            ts(B2, B2, 1e-12, OP.max)
            act_sqrt(DA, A2)
            act_sqrt(DB, B2)
            tt(CR, DA, DB, OP.subtract)
            tt(CR, CR, CR, OP.mult)   # |da-db|^2 == (da-db)^2 exactly
            ts(M144, CR, -float(INV_T2), OP.mult, 1.0, OP.add)
            ts(M144, M144, 0.0, OP.max)
            for i in range(12):
                mset(M144[:, 13 * i:13 * i + 1], 0.0)
            if debug:
                cp(dbg_sb["dm"], M144)

            # ---- power iteration (10 iters) ----
            m3 = M144.rearrange("p (i j) -> p i j", i=12)
            VN = alloc(12); T12 = alloc(12)
            N2 = alloc(1); NN = alloc(1); RN = alloc(1)
            mset(VV, 1.0)
            for _ in range(10):
                tt(PR.rearrange("p (i j) -> p i j", i=12), m3,
                   VV.unsqueeze(1).to_broadcast([P, 12, 12]), OP.mult)
                red(VN, PR.rearrange("p (i j) -> p i j", i=12))
                tt(T12, VN, VN, OP.mult)
                red(N2, T12)
                act_sqrt(NN, N2)
                ts(NN, NN, 1e-6, OP.add)
                rcp(RN, NN)
                ts(VV, VN, RN, OP.mult)
            if debug:
                cp(dbg_sb["dvv"], VV)
            # w = v / (sum(v) + 1e-6)
            S1 = alloc(1); RS = alloc(1)
            red(S1, VV)
            ts(S1, S1, 1e-6, OP.add)
            rcp(RS, S1)
            ts(WW, VV, RS, OP.mult)
            if debug:
                cp(dbg_sb["dww"], WW)

            # ---- weighted Kabsch ----
            a3 = curx[:, :36].rearrange("p (c b) -> p c b", c=3)
            b3 = cury[:, :36].rearrange("p (c b) -> p c b", c=3)
            WS = alloc(1); RWS = alloc(1)
            red(WS, WW)
            ts(WS, WS, 1e-6, OP.add)
            rcp(RWS, WS)
            WA = alloc(36); SA = alloc(3); CA = alloc(3); CB = alloc(3)
            AM = alloc(36); BM = alloc(36); WAM = alloc(36)
            wb = WW.unsqueeze(1).to_broadcast([P, 3, 12])
            tt(WA.rearrange("p (c b) -> p c b", c=3), a3, wb, OP.mult)
            red(SA, WA.rearrange("p (c b) -> p c b", c=3))
            ts(CA, SA, RWS, OP.mult)
            tt(WA.rearrange("p (c b) -> p c b", c=3), b3, wb, OP.mult)
            red(SA, WA.rearrange("p (c b) -> p c b", c=3))
            ts(CB, SA, RWS, OP.mult)
            tt(AM.rearrange("p (c b) -> p c b", c=3), a3,
               CA.unsqueeze(2).to_broadcast([P, 3, 12]), OP.subtract)
            tt(BM.rearrange("p (c b) -> p c b", c=3), b3,
               CB.unsqueeze(2).to_broadcast([P, 3, 12]), OP.subtract)
            tt(WAM.rearrange("p (c b) -> p c b", c=3),
               AM.rearrange("p (c b) -> p c b", c=3), wb, OP.mult)
            HP = alloc(108); H9 = alloc(9)
            tt(HP.rearrange("p (i j b) -> p i j b", i=3, j=3),
               WAM.rearrange("p (c b) -> p c b", c=3).unsqueeze(2).to_broadcast([P, 3, 3, 12]),
               BM.rearrange("p (c b) -> p c b", c=3).unsqueeze(1).to_broadcast([P, 3, 3, 12]),
               OP.mult)
            red(H9, HP.rearrange("p (i j b) -> p i j b", i=3, j=3))
            if debug:
                cp(dbg_sb["dh9"], H9)
            KP = alloc(27); K9 = alloc(9)
            h3v = H9.rearrange("p (i j) -> p i j", i=3)
            tt(KP.rearrange("p (i l j) -> p i l j", i=3, l=3),
               h3v.unsqueeze(2).to_broadcast([P, 3, 3, 3]),
               h3v.unsqueeze(1).to_broadcast([P, 3, 3, 3]), OP.mult)
            red(K9, KP.rearrange("p (i l j) -> p i l j", i=3, l=3))
            if debug:
                cp(dbg_sb["dk9"], K9)

            # ---- closed-form eigenvalues of K (3x3 sym PSD) ----
            c1_ = lambda i: K9[:, i:i + 1]
            QQ = alloc(1)
            tt(QQ, c1_(0), c1_(4), OP.add)
            tt(QQ, QQ, c1_(8), OP.add)
            ts(QQ, QQ, float(F32(1.0 / 3.0)), OP.mult)
            KD = alloc(3)   # K00-qq, K11-qq, K22-qq
            for di, src_i in enumerate((0, 4, 8)):
                tt(KD[:, di:di + 1], c1_(src_i), QQ, OP.subtract)
            P1 = alloc(1); TTa = alloc(1); TTb = alloc(1)
            tt(P1, c1_(1), c1_(1), OP.mult)
            tt(TTa, c1_(2), c1_(2), OP.mult)
            tt(P1, P1, TTa, OP.add)
            tt(TTa, c1_(5), c1_(5), OP.mult)
            tt(P1, P1, TTa, OP.add)
            P2 = alloc(1)
            tt(P2, KD[:, 0:1], KD[:, 0:1], OP.mult)
            tt(TTa, KD[:, 1:2], KD[:, 1:2], OP.mult)
            tt(P2, P2, TTa, OP.add)
            tt(TTa, KD[:, 2:3], KD[:, 2:3], OP.mult)
            tt(P2, P2, TTa, OP.add)
            ts(TTa, P1, 2.0, OP.mult)
            tt(P2, P2, TTa, OP.add)
            PV = alloc(1); RP = alloc(1)
            ts(PV, P2, float(F32(1.0 / 6.0)), OP.mult)
            act_sqrt(PV, PV)
            ts(TTa, PV, 1e-30, OP.max)
            rcp(RP, TTa)
            BV = alloc(6)   # B00,B11,B22,B01,B02,B12
            for bi_, src in enumerate((KD[:, 0:1], KD[:, 1:2], KD[:, 2:3],
                                       c1_(1), c1_(2), c1_(5))):
                ts(BV[:, bi_:bi_ + 1], src, RP, OP.mult)
            B00, B11, B22 = BV[:, 0:1], BV[:, 1:2], BV[:, 2:3]
            B01, B02, B12 = BV[:, 3:4], BV[:, 4:5], BV[:, 5:6]
            DET = alloc(1); TTc = alloc(1)
            # t1 = B00*(B11*B22 - B12*B12)
            tt(TTa, B11, B22, OP.mult)
            tt(TTb, B12, B12, OP.mult)
            tt(TTa, TTa, TTb, OP.subtract)
            tt(DET, B00, TTa, OP.mult)
            # t2 = B01*(B01*B22 - B12*B02); det = t1 - t2
            tt(TTa, B01, B22, OP.mult)
            tt(TTb, B12, B02, OP.mult)
            tt(TTa, TTa, TTb, OP.subtract)
            tt(TTc, B01, TTa, OP.mult)
            tt(DET, DET, TTc, OP.subtract)
            # t3 = B02*(B01*B12 - B11*B02); det = det + t3
            tt(TTa, B01, B12, OP.mult)
            tt(TTb, B11, B02, OP.mult)
            tt(TTa, TTa, TTb, OP.subtract)
            tt(TTc, B02, TTa, OP.mult)
            tt(DET, DET, TTc, OP.add)
            RV = alloc(1)
            ts(RV, DET, 0.5, OP.mult)
            ts(RV, RV, -1.0, OP.max)
            ts(RV, RV, 1.0, OP.min)
            CC = alloc(1); C2 = alloc(1); C3 = alloc(1)
            FF = alloc(1); FP = alloc(1); RFP = alloc(1)
            mset(CC, 1.0)
            for _ in range(6):
                tt(C2, CC, CC, OP.mult)
                tt(C3, C2, CC, OP.mult)
                ts(FF, C3, 4.0, OP.mult)
                ts(TTa, CC, 3.0, OP.mult)
                tt(FF, FF, TTa, OP.subtract)
                tt(FF, FF, RV, OP.subtract)
                ts(FP, C2, 12.0, OP.mult, 3.0, OP.subtract)
                ts(FP, FP, 1e-6, OP.max)
                rcp(RFP, FP)
                tt(TTa, FF, RFP, OP.mult)
                tt(CC, CC, TTa, OP.subtract)
                ts(CC, CC, 0.5, OP.max)
                ts(CC, CC, 1.0, OP.min)
            SS = alloc(1)
            tt(SS, CC, CC, OP.mult)
            ts(SS, SS, -1.0, OP.mult, 1.0, OP.add)
            ts(SS, SS, 0.0, OP.max)
            act_sqrt(SS, SS)
            LAM1 = alloc(1); LAM2 = alloc(1)
            ts(TTa, PV, 2.0, OP.mult)
            tt(TTa, TTa, CC, OP.mult)
            tt(LAM1, QQ, TTa, OP.add)
            ts(TTa, CC, -0.5, OP.mult)
            ts(TTb, SS, float(F32(np.sqrt(3.0) / 2.0)), OP.mult)
            tt(TTa, TTa, TTb, OP.add)
            ts(TTb, PV, 2.0, OP.mult)
            tt(TTa, TTa, TTb, OP.mult)
            tt(LAM2, QQ, TTa, OP.add)
            if debug:
                cp(dbg_sb["dlam"][:, 0:1], LAM1)
                cp(dbg_sb["dlam"][:, 1:2], LAM2)

            # ---- eigenvectors ----
            AK = alloc(9)
            C1v = alloc(3); C2v = alloc(3); C3v = alloc(3)
            N1 = alloc(1); N2e = alloc(1); N3e = alloc(1)
            MA = alloc(1); MB = alloc(1); MC = alloc(1)
            T3v = alloc(3); NU = alloc(1); NS = alloc(1); RNU = alloc(1)
            U1 = alloc(3); U2 = alloc(3); U3 = alloc(3)

            def eigvec(lam, uout):
                cp(AK, K9)
                for d in range(3):
                    tt(AK[:, 4 * d:4 * d + 1], AK[:, 4 * d:4 * d + 1], lam, OP.subtract)
                r0, r1, r2 = AK[:, 0:3], AK[:, 3:6], AK[:, 6:9]
                cross3(C1v, r0, r1, TTa, TTb)
                cross3(C2v, r1, r2, TTa, TTb)
                cross3(C3v, r2, r0, TTa, TTb)
                for (cv, nv) in ((C1v, N1), (C2v, N2e), (C3v, N3e)):
                    tt(T3v, cv, cv, OP.mult)
                    red(nv, T3v)
                tt(MA, N1, N2e, OP.is_ge)
                tt(TTa, N1, N3e, OP.is_ge)
                tt(MA, MA, TTa, OP.mult)
                ts(TTa, MA, -1.0, OP.mult, 1.0, OP.add)     # 1 - a1
                tt(MB, N2e, N3e, OP.is_ge)
                tt(MB, TTa, MB, OP.mult)                     # a2
                tt(MC, TTa, MB, OP.subtract)                 # a3
                ts(uout, C1v, MA, OP.mult)
                ts(T3v, C2v, MB, OP.mult)
                tt(uout, uout, T3v, OP.add)
                ts(T3v, C3v, MC, OP.mult)
                tt(uout, uout, T3v, OP.add)
                normalize3(uout, NU, NS, RNU, T3v)

            eigvec(LAM1, U1)
            eigvec(LAM2, U2)
            if debug:
                cp(dbg_sb["du1"], U1)
            # Gram-Schmidt u2 against u1
            DOT = alloc(1)
            tt(T3v, U1, U2, OP.mult)
            red(DOT, T3v)
            ts(T3v, U1, DOT, OP.mult)
            tt(U2, U2, T3v, OP.subtract)
            normalize3(U2, NU, NS, RNU, T3v)
            if debug:
                cp(dbg_sb["du2"], U2)
            cross3(U3, U1, U2, TTa, TTb)

            # v_i = normalize(H^T u_i); v3 = v1 x v2
            HP2 = alloc(9)
            V1 = alloc(3); V2 = alloc(3); V3 = alloc(3)
            ht3 = H9.rearrange("p (i j) -> p i j", i=3).transpose([0, 2, 1])
            for (uin, vout) in ((U1, V1), (U2, V2)):
                tt(HP2.rearrange("p (i j) -> p i j", i=3), ht3,
                   uin.unsqueeze(1).to_broadcast([P, 3, 3]), OP.mult)
                red(vout, HP2.rearrange("p (i j) -> p i j", i=3))
                normalize3(vout, NU, NS, RNU, T3v)
            if debug:
                cp(dbg_sb["dv1"], V1)
            cross3(V3, V1, V2, TTa, TTb)

            # R = v1 u1^T + v2 u2^T + v3 u3^T ;  t = cB - R cA
            OP9 = alloc(9)
            tt(R9S.rearrange("p (i j) -> p i j", i=3),
               V1.unsqueeze(2).to_broadcast([P, 3, 3]),
               U1.unsqueeze(1).to_broadcast([P, 3, 3]), OP.mult)
            for (vv_, uu_) in ((V2, U2), (V3, U3)):
                tt(OP9.rearrange("p (i j) -> p i j", i=3),
                   vv_.unsqueeze(2).to_broadcast([P, 3, 3]),
                   uu_.unsqueeze(1).to_broadcast([P, 3, 3]), OP.mult)
                tt(R9S, R9S, OP9, OP.add)
            tt(OP9.rearrange("p (i j) -> p i j", i=3),
               R9S.rearrange("p (i j) -> p i j", i=3),
               CA.unsqueeze(1).to_broadcast([P, 3, 3]), OP.mult)
            RC = alloc(3)
            red(RC, OP9.rearrange("p (i j) -> p i j", i=3))
            tt(T3S, CB, RC, OP.subtract)

            if trunc == 3:
                fence()
                return
            # ---- fitness: count ||R x + t - y|| < 0.1 over all 2048 pts ----
            DC = SCR[:, 0:6144].rearrange("p (c n) -> p c n", c=3)
            ACC = SCR[:, 6144:6144 + 2048]
            L2S = SCR[:, 8192:8192 + 2048]
            SQ = SCR[:, 10240:10240 + 2048]
            xv = PSRC.rearrange("p (c n) -> p c n", c=3)
            yv = PTGT.rearrange("p (c n) -> p c n", c=3)
            for c in range(3):
                ts(ACC, xv[:, 0, :], R9S[:, 3 * c:3 * c + 1], OP.mult,
                   T3S[:, c:c + 1], OP.add)
                stt(ACC, xv[:, 1, :], R9S[:, 3 * c + 1:3 * c + 2], ACC, OP.mult, OP.add)
                stt(ACC, xv[:, 2, :], R9S[:, 3 * c + 2:3 * c + 3], ACC, OP.mult, OP.add)
                tt(DC[:, c, :], ACC, yv[:, c, :], OP.subtract)
            tt(L2S, DC[:, 0, :], DC[:, 0, :], OP.mult)
            tt(SQ, DC[:, 1, :], DC[:, 1, :], OP.mult)
            tt(L2S, L2S, SQ, OP.add)
            tt(SQ, DC[:, 2, :], DC[:, 2, :], OP.mult)
            tt(L2S, L2S, SQ, OP.add)
            ts(SQ, L2S, float(T2), OP.is_lt)
            red(CNTS, SQ)
            fence()

        @block.scalar
        def _(scalar):
            from concourse import mybir as mb
            for (vt, src, dst) in sqrt_jobs:
                scalar.wait_ge(vsem, vt)
                nc.scalar.sqrt(dst, src).then_inc(asem, 1)

        @block.gpsimd
        def _(gpsimd):
            gpsimd.dma_start(INP, inp[:, :]).then_inc(dins, 16)
            gpsimd.iota(POSI, pattern=[[1, 200]], base=0,
                        channel_multiplier=0).then_inc(gsem, 1)
            # rebuild replicated keypoint rows: scatter -> DRAM -> broadcast
            gpsimd.wait_ge(dins, 16)
            gpsimd.dma_start(dscr[0:1, :].rearrange("p (a b) -> p a b", a=P),
                             INP[:, 200:392]).then_inc(dpts, 16)
            gpsimd.wait_ge(dpts, 16)
            gpsimd.dma_start(PSRC, dscr[0:1, 0:3 * NPTS].to_broadcast([P, 3 * NPTS])).then_inc(dpts, 16)
            gpsimd.dma_start(PTGT, dscr[0:1, 3 * NPTS:6 * NPTS].to_broadcast([P, 3 * NPTS])).then_inc(dpts, 16)
            gpsimd.wait_ge(vsem, vcnt[0])
            nout = 1 + len(dbg_names)
            gpsimd.dma_start(out13[:, :], OUT13).then_inc(dout, 16)
            if debug:
                for n_ in dbg_names:
                    gpsimd.dma_start(dbg_dram[n_][:, :], dbg_sb[n_]).then_inc(dout, 16)
            gpsimd.wait_ge(dout, 16 * nout)
    return nc


def _get_prog(key, builder):
    if key not in _programs:
        _programs[key] = builder()
    return _programs[key]


_cache_cfg = []


def _enable_jax_cache():
    if _cache_cfg:
        return
    _cache_cfg.append(1)
    try:
        import jax
        jax.config.update("jax_compilation_cache_dir", "/tmp/jax_comp_cache")
        jax.config.update("jax_persistent_cache_min_compile_time_secs", 0)
        jax.config.update("jax_persistent_cache_min_entry_size_bytes", 0)
    except Exception:
        pass


def _run(nc, in_maps):
    import time
    _enable_jax_cache()
    from concourse.bass_utils import run_bass_kernel_spmd
    last = None
    for attempt in range(3):
        try:
            t0 = time.time()
            res = run_bass_kernel_spmd(nc, in_maps, core_ids=list(range(NCORES)))
            _launch_wall.append(time.time() - t0)
            return res.results
        except Exception as e:  # transient device errors: retry
            last = e
    raise last


def kernel(SC2_measure, src_keypts, tgt_keypts):
    _launch_wall.clear()
    SC2 = np.ascontiguousarray(SC2_measure[0], dtype=np.float32)      # [512, 2048]
    src = np.ascontiguousarray(src_keypts[0], dtype=np.float32)       # [2048, 3]
    tgt = np.ascontiguousarray(tgt_keypts[0], dtype=np.float32)

    # exact top-200 per seed (desc value, ties -> lower index == lax.top_k)
    knn = np.argsort(-SC2, axis=1, kind='stable')[:, :200]
    knnf = knn.astype(np.float32)                                     # ints < 2048, exact
    ptsflat = np.concatenate([src.T.reshape(3 * NPTS), tgt.T.reshape(3 * NPTS)])
    ptsrows = ptsflat.reshape(SPC, 192)                               # row r: flat[192r:192r+192]

    nc = _get_prog("mega", _prog_mega)
    in_maps = []
    for c in range(NCORES):
        m = np.empty((SPC, 392), np.float32)
        m[:, 0:200] = knnf[c * SPC:(c + 1) * SPC]
        m[:, 200:392] = ptsrows
        in_maps.append({"inp": m})
    for _try in range(3):
        res = _run(nc, in_maps)
        o = np.concatenate([res[c]["out13"] for c in range(NCORES)])  # [512,13]
        cntv = o[:, 0]
        R = o[:, 1:10].reshape(SEEDS, 3, 3)
        t = o[:, 10:13]
        ok = (np.isfinite(cntv).all() and (cntv == np.round(cntv)).all()
              and (cntv >= 0).all() and (cntv <= NPTS).all()
              and np.isfinite(R).all() and np.isfinite(t).all())
        if ok:
            break
    best = int(np.argmax(cntv))
    T = np.zeros((1, 4, 4), np.float32)
    T[0, :3, :3] = R[best]
    T[0, :3, 3] = t[best]
    T[0, 3, 3] = 1.0
    return T


# revision 67
# speedup vs baseline: 1.4236x; 1.0746x over previous
"""Trainium2 Bass kernel for nn_HCF_module (SC2 NMS/registration pipeline).

Single fused device launch (512 seeds sharded 64/core over 8 NeuronCores,
keypoints replicated). Host does only the exact top-200 seed-row selection
(stable argsort = lax.top_k tie order), ships one packed [64, 392] input
per core (200 indices + keypoint scatter rows), and does the final
argmax/T assembly. One input + one output dram tensor per core — per-array
RPC overhead (~40ms/array) dominates the launch wall on axon.

Device program per core (64 seeds on 64 partitions):
  - replicate keypoints to all partitions (scatter rows -> Internal DRAM
    -> stride-0 broadcast DMA), then gather each seed's top-200 coords
    via exact one-hot is_eq over a device-built 0..2047 iota.
  - 4 filter stages k=200/100/50/25: SC2 consistency scores (sqrt-free
    hard-bit test, bit-identical to the validated baseline arithmetic),
    then EXACT top-k/2 selection via unique integer keys 256*sc2 - pos
    (f32-exact integers; DVE max8/match_replace rounds), then one-hot
    is_eq gather of the selected neighbor coords (exact f32 copies).
  - tail: local_sc matrix, 10x power iteration, weighted Kabsch via
    closed-form 3x3 eigensolver + Newton (same op order as the validated
    f32 host model), fitness inlier counts over all 2048 keypoints.
Outputs per seed (packed [64, 13]): inlier count, R (3x3), t (3).
"""
import os as _os
import numpy as np

# Persistent XLA compilation cache: the PJRT wrapper is re-traced per launch
# (fresh closure inside run_bass_kernel_spmd), so without this every warm
# launch re-runs the BIR->NEFF backend pipeline (~0.3s). Must be set before
# jax initializes.
_os.environ.setdefault("JAX_COMPILATION_CACHE_DIR", "/tmp/jax_comp_cache")
_os.environ.setdefault("JAX_PERSISTENT_CACHE_MIN_COMPILE_TIME_SECS", "0")
_os.environ.setdefault("JAX_PERSISTENT_CACHE_MIN_ENTRY_SIZE_BYTES", "0")

F32 = np.float32
T2 = F32(0.1) * F32(0.1)            # 0.010000000707...
TWO_T2 = F32(2.0) * T2
T4 = T2 * T2
INV_T2 = F32(np.float64(1.0) / np.float64(T2))
NCORES = 8
SEEDS = 512
SPC = SEEDS // NCORES               # seeds per core
NPTS = 2048
NEG = -1e30

# filter stages: (k, B, kf, gather-chunk mc)
STAGES = [(200, 20, 100, 50), (100, 20, 50, 50), (50, 25, 25, 25), (25, 25, 12, 12)]

_programs = {}
_launch_wall = []


def _mk_bass(detect_races=True):
    import concourse.bass as bass
    return bass.Bass("TRN2", target_bir_lowering=False,
                     detect_race_conditions=detect_races)


def _prog_mega(debug=False, sync_all=True, trunc=0):
    """Build the fused device program.

    sync_all=True emits a vsem inc+wait after every DVE instruction —
    required by CoreSim's race model (used for validation builds).
    sync_all=False relies on in-order engine execution with the HW's
    per-op pipeline drain, fencing only at ACT/DMA crossings (faster).
    """
    import concourse.mybir as mybir
    from concourse.alu_op_type import AluOpType as OP
    nc = _mk_bass(detect_races=sync_all)
    P = SPC
    # single packed input per core: [0:200) top-200 knn indices (f32 integers)
    # | [200:392) keypts scatter (row r holds pts.flat[r*192:(r+1)*192],
    # pts.flat = src c-major 6144 floats then tgt c-major 6144 floats)
    inp = nc.dram_tensor("inp", [P, 392], mybir.dt.float32, kind="ExternalInput")
    dscr = nc.dram_tensor("dscr", [1, 2 * 3 * NPTS], mybir.dt.float32, kind="Internal")
    # single packed output: col 0 cnt | 1:10 R row-major | 10:13 t
    out13 = nc.dram_tensor("out13", [P, 13], mybir.dt.float32, kind="ExternalOutput")
    dbg_names = []
    if debug:
        dbg_specs = [("dsc1", 200), ("dsc2", 100), ("dsc3", 50), ("dsc4", 25),
                     ("dxf", 36), ("dyf", 36), ("dm", 144),
                     ("dvv", 12), ("dww", 12), ("dh9", 9), ("dk9", 9),
                     ("dlam", 2), ("du1", 3), ("du2", 3), ("dv1", 3)]
        dbg_dram = {n: nc.dram_tensor(n, [P, w], mybir.dt.float32, kind="ExternalOutput")
                    for (n, w) in dbg_specs}
        dbg_names = [n for (n, _) in dbg_specs]

    ctx = nc.ctx
    sb = lambda nm, shape: ctx.enter_context(nc.sbuf_tensor(nm, shape, mybir.dt.float32))[:, :]
    INP = sb("INP", [P, 392])
    IDX = INP[:, 0:200]
    POSI = ctx.enter_context(nc.sbuf_tensor("POSI", [P, 200], mybir.dt.int32))[:, :]
    POS = sb("POS", [P, 200])
    TXa = sb("TXa", [P, 600]); TYa = sb("TYa", [P, 600])
    TXb = sb("TXb", [P, 304]); TYb = sb("TYb", [P, 304])
    TXc = sb("TXc", [P, 304]); TYc = sb("TYc", [P, 304])
    SC2S = sb("SC2S", [P, 200]); H0 = sb("H0", [P, 200])
    KEYP = sb("KEYP", [P, 200]); KEYW = sb("KEYW", [P, 200]); TOPV = sb("TOPV", [P, 104])
    PSRC = sb("PSRC", [P, 3 * NPTS]); PTGT = sb("PTGT", [P, 3 * NPTS])
    VV = sb("VV", [P, 12]); WW = sb("WW", [P, 12])
    OUT13 = sb("OUT13", [P, 13])
    CNTS = OUT13[:, 0:1]; R9S = OUT13[:, 1:10]; T3S = OUT13[:, 10:13]
    FEN = sb("FEN", [P, 1])
    SCR = sb("SCR", [P, 36000])
    IOTA2K = SCR[:, 32768:32768 + NPTS]   # live only during gather_top200
    if debug:
        dbg_sb = {n: sb("sb_" + n, [P, w]) for (n, w) in dbg_specs}

    dins = ctx.enter_context(nc.semaphore())
    dpts = ctx.enter_context(nc.semaphore())
    dout = ctx.enter_context(nc.semaphore())
    vsem = ctx.enter_context(nc.semaphore())
    asem = ctx.enter_context(nc.semaphore())
    gsem = ctx.enter_context(nc.semaphore())

    vcnt = [0]
    acnt = [0]
    sqrt_jobs = []   # (vsem threshold, src AP, dst AP)
    veng = [None]
    marks = {}

    def V(inst):
        # embed the order-edge in the instruction itself: wait for the
        # previous instruction's vsem value, inc after completion. Same
        # fence semantics as a standalone wait, half the BIR entries.
        if sync_all:
            if vcnt[0] > 0:
                inst.wait_op(vsem, vcnt[0], "sem-ge")
            inst.then_inc(vsem, 1)
            vcnt[0] += 1
        return inst

    def fence():
        # make vsem reflect completion of all vector work so far
        if not sync_all:
            nc.vector.tensor_copy(FEN, FEN).then_inc(vsem, 1)
            vcnt[0] += 1

    def tt(out, a, b, op):
        V(nc.vector.tensor_tensor(out=out, in0=a, in1=b, op=op))

    def ts(out, a, s1, op0, s2=None, op1=None):
        if op1 is None:
            V(nc.vector.tensor_scalar(out, a, s1, None, op0))
        else:
            V(nc.vector.tensor_scalar(out, a, s1, s2, op0, op1))

    def stt(out, in0, s, in1, op0, op1):
        V(nc.vector.scalar_tensor_tensor(out=out, in0=in0, scalar=s, in1=in1,
                                         op0=op0, op1=op1))

    def cp(out, a):
        V(nc.vector.tensor_copy(out, a))

    def red(out, in_, op=None):
        V(nc.vector.tensor_reduce(out=out, in_=in_, axis=mybir.AxisListType.X,
                                  op=op or OP.add))

    def mset(ap, v):
        V(nc.vector.memset(ap, v))

    def rcp(out, in_):
        V(nc.vector.reciprocal(out, in_))

    def act_sqrt(dst, src):
        fence()
        sqrt_jobs.append((vcnt[0], src, dst))
        acnt[0] += 1
        veng[0].wait_ge(asem, acnt[0])

    def sc2_stage(k, B, tx, ty):
        slot = 4000 if k == 200 else 2000
        dxs = SCR[:, 0:B * 3 * k]
        d2a = SCR[:, 12000:12000 + B * k]
        d2b = SCR[:, 12000 + slot:12000 + slot + B * k]
        q = SCR[:, 12000 + 2 * slot:12000 + 2 * slot + B * k]
        pp = SCR[:, 12000 + 3 * slot:12000 + 3 * slot + B * k]
        hard = SCR[:, 12000 + 4 * slot:12000 + 4 * slot + B * k]
        scr2 = SCR[:, 12000 + 5 * slot:12000 + 5 * slot + B * k]
        nb = k // B
        for bi in range(nb):
            a0 = bi * B
            for (src_t, dst) in ((tx, d2a), (ty, d2b)):
                v3 = src_t[:, :3 * k].rearrange("p (c b) -> p c b", c=3)
                rows4 = v3.unsqueeze(1).to_broadcast([P, B, 3, k])
                cols4 = v3[:, :, a0:a0 + B].transpose([0, 2, 1]).unsqueeze(3).to_broadcast([P, B, 3, k])
                dx4 = dxs.rearrange("p (a c b) -> p a c b", a=B, c=3)
                tt(dx4, rows4, cols4, OP.subtract)
                tt(dxs, dxs, dxs, OP.mult)
                d2v = dst.rearrange("p (a b) -> p a b", a=B)
                tt(d2v, dx4[:, :, 0, :], dx4[:, :, 1, :], OP.add)
                tt(d2v, d2v, dx4[:, :, 2, :], OP.add)
            tt(q, d2a, d2b, OP.add)
            tt(pp, d2a, d2b, OP.subtract)
            tt(pp, pp, pp, OP.mult)
            ts(scr2, q, float(TWO_T2), OP.mult, float(T4), OP.subtract)
            tt(hard, pp, scr2, OP.is_lt)
            ts(scr2, q, float(T2), OP.is_lt)
            tt(hard, hard, scr2, OP.max)
            if bi == 0:
                cp(H0[:, :k], hard[:, :k])
            hv = hard.rearrange("p (a b) -> p a b", a=B)
            h0c = H0[:, a0:a0 + B].unsqueeze(2).to_broadcast([P, B, k])
            tt(hv, hv, h0c, OP.mult)
            hT = hv.transpose([0, 2, 1])
            if bi == 0:
                red(SC2S[:, :k], hT)
            else:
                red(scr2[:, :k], hT)
                tt(SC2S[:, :k], SC2S[:, :k], scr2[:, :k], OP.add)

    def key_topk(k, kf):
        # unique integer keys: 256*sc2 - pos; desc key order == (sc2 desc, pos asc)
        ts(KEYP[:, :k], SC2S[:, :k], 256.0, OP.mult)
        tt(KEYP[:, :k], KEYP[:, :k], POS[:, :k], OP.subtract)
        cp(KEYW[:, :k], KEYP[:, :k])
        rounds = (kf + 7) // 8
        for r in range(rounds):
            V(nc.vector.max(out=TOPV[:, r * 8:(r + 1) * 8], in_=KEYW[:, :k]))
            if r < rounds - 1:
                V(nc.vector.match_replace(out=KEYW[:, :k],
                                          in_to_replace=TOPV[:, r * 8:(r + 1) * 8],
                                          in_values=KEYW[:, :k], imm_value=NEG))

    def gather_top200():
        # TXa/TYa[:, c*200+m] = keypts[idx[m], c] via exact one-hot over 2048
        cp(POS, POSI)   # int32 -> f32, exact for 0..199 (iota runs on gpsimd)
        for c in range(10):
            ts(IOTA2K[:, c * 200:(c + 1) * 200], POS, float(200 * c), OP.add)
        ts(IOTA2K[:, 2000:2048], POS[:, 0:48], 2000.0, OP.add)
        mcg = 8
        pv3 = PSRC.rearrange("p (c n) -> p c n", c=3)
        tv3 = PTGT.rearrange("p (c n) -> p c n", c=3)
        for c0 in range(0, 200, mcg):
            w = min(mcg, 200 - c0)
            oh3 = SCR[:, 0:w * NPTS].rearrange("p (m j) -> p m j", m=w)
            tmp3 = SCR[:, mcg * NPTS:mcg * NPTS + w * NPTS].rearrange("p (m j) -> p m j", m=w)
            sel = IDX[:, c0:c0 + w]
            tt(oh3, sel.unsqueeze(2).to_broadcast([P, w, NPTS]),
               IOTA2K.unsqueeze(1).to_broadcast([P, w, NPTS]), OP.is_equal)
            for (src3, t_out) in ((pv3, TXa), (tv3, TYa)):
                for c in range(3):
                    tt(tmp3, oh3,
                       src3[:, c, :].unsqueeze(1).to_broadcast([P, w, NPTS]),
                       OP.mult)
                    red(t_out[:, c * 200 + c0:c * 200 + c0 + w], tmp3)

    def gather(k, kf, mc, tx, ty, ox, oy):
        oh3 = SCR[:, 0:mc * k].rearrange("p (m j) -> p m j", m=mc)
        tmp3 = SCR[:, mc * k:2 * mc * k].rearrange("p (m j) -> p m j", m=mc)
        for c0 in range(0, kf, mc):
            sel = TOPV[:, c0:c0 + mc]
            tt(oh3, sel.unsqueeze(2).to_broadcast([P, mc, k]),
               KEYP[:, :k].unsqueeze(1).to_broadcast([P, mc, k]), OP.is_equal)
            for (t_in, t_out) in ((tx, ox), (ty, oy)):
                for c in range(3):
                    tt(tmp3, oh3,
                       t_in[:, c * k:(c + 1) * k].unsqueeze(1).to_broadcast([P, mc, k]),
                       OP.mult)
                    red(t_out[:, c * kf + c0:c * kf + c0 + mc], tmp3)

    scr_off = [0]

    def alloc(n):
        off = scr_off[0]
        scr_off[0] += n
        assert scr_off[0] <= 12000
        return SCR[:, off:off + n]

    def cross3(out, a, b, tA, tB):
        for i in range(3):
            j, kk = (i + 1) % 3, (i + 2) % 3
            tt(tA, a[:, j:j + 1], b[:, kk:kk + 1], OP.mult)
            tt(tB, a[:, kk:kk + 1], b[:, j:j + 1], OP.mult)
            tt(out[:, i:i + 1], tA, tB, OP.subtract)

    def normalize3(u, nu, ns, rn, t3v, eps=1e-38):
        # u *= 1/sqrt(max(sum(u^2), eps))
        tt(t3v, u, u, OP.mult)
        red(nu, t3v)
        ts(nu, nu, eps, OP.max)
        act_sqrt(ns, nu)
        rcp(rn, ns)
        ts(u, u, rn, OP.mult)

    with nc.Block() as block:
        @block.vector
        def _(vector):
            veng[0] = vector
            mset(FEN, 0.0)
            vector.wait_ge(dins, 16)     # INP DMA
            vector.wait_ge(gsem, 1)      # gpsimd iota
            vector.wait_ge(dpts, 48)     # PSRC/PTGT replicated
            gather_top200()
            curx, cury = TXa, TYa
            for si, (k, B, kf, mc) in enumerate(STAGES):
                nxtx, nxty = (TXb, TYb) if si % 2 == 0 else (TXc, TYc)
                sc2_stage(k, B, curx, cury)
                if trunc == 1 and si == 0:
                    fence()
                    return
                if debug:
                    cp(dbg_sb[["dsc1", "dsc2", "dsc3", "dsc4"][si]], SC2S[:, :k])
                key_topk(k, kf)
                gather(k, kf, mc, curx, cury, nxtx, nxty)
                curx, cury = nxtx, nxty
            if trunc == 2:
                fence()
                return
            # final selected coords: curx[:, :36], cury[:, :36] (c-major, 12 each)
            if debug:
                cp(dbg_sb["dxf"], curx[:, :36])
                cp(dbg_sb["dyf"], cury[:, :36])

            # ---- local_sc matrix M [12x12] ----
            DX = alloc(432)
            A2 = alloc(144); B2 = alloc(144)
            DA = alloc(144); DB = alloc(144)
            CR = alloc(144); M144 = alloc(144); PR = alloc(144)
            for (t_in, d2out) in ((curx, A2), (cury, B2)):
                v3 = t_in[:, :36].rearrange("p (c b) -> p c b", c=3)
                rows4 = v3.unsqueeze(1).to_broadcast([P, 12, 3, 12])
                cols4 = v3.transpose([0, 2, 1]).unsqueeze(3).to_broadcast([P, 12, 3, 12])
                dx4 = DX.rearrange("p (a c b) -> p a c b", a=12, c=3)
                tt(dx4, rows4, cols4, OP.subtract)
                tt(DX, DX, DX, OP.mult)
                d2v = d2out.rearrange("p (a b) -> p a b", a=12)
                tt(d2v, dx4[:, :, 0, :], dx4[:, :, 1, :], OP.add)
                tt(d2v, d2v, dx4[:, :, 2, :], OP.add)
            ts(A2, A2, 1e-12, OP.max)
            ts(B2, B2, 1e-12, OP.max)
            act_sqrt(DA, A2)
            act_sqrt(DB, B2)
            tt(CR, DA, DB, OP.subtract)
            tt(CR, CR, CR, OP.mult)   # |da-db|^2 == (da-db)^2 exactly
            ts(M144, CR, -float(INV_T2), OP.mult, 1.0, OP.add)
            ts(M144, M144, 0.0, OP.max)
            for i in range(12):
                mset(M144[:, 13 * i:13 * i + 1], 0.0)
            if debug:
                cp(dbg_sb["dm"], M144)

            # ---- power iteration (10 iters) ----
            m3 = M144.rearrange("p (i j) -> p i j", i=12)
            VN = alloc(12); T12 = alloc(12)
            N2 = alloc(1); NN = alloc(1); RN = alloc(1)
            mset(VV, 1.0)
            for _ in range(10):
                tt(PR.rearrange("p (i j) -> p i j", i=12), m3,
                   VV.unsqueeze(1).to_broadcast([P, 12, 12]), OP.mult)
                red(VN, PR.rearrange("p (i j) -> p i j", i=12))
                tt(T12, VN, VN, OP.mult)
                red(N2, T12)
                act_sqrt(NN, N2)
                ts(NN, NN, 1e-6, OP.add)
                rcp(RN, NN)
                ts(VV, VN, RN, OP.mult)
            if debug:
                cp(dbg_sb["dvv"], VV)
            # w = v / (sum(v) + 1e-6)
            S1 = alloc(1); RS = alloc(1)
            red(S1, VV)
            ts(S1, S1, 1e-6, OP.add)
            rcp(RS, S1)
            ts(WW, VV, RS, OP.mult)
            if debug:
                cp(dbg_sb["dww"], WW)

            # ---- weighted Kabsch ----
            a3 = curx[:, :36].rearrange("p (c b) -> p c b", c=3)
            b3 = cury[:, :36].rearrange("p (c b) -> p c b", c=3)
            WS = alloc(1); RWS = alloc(1)
            red(WS, WW)
            ts(WS, WS, 1e-6, OP.add)
            rcp(RWS, WS)
            WA = alloc(36); SA = alloc(3); CA = alloc(3); CB = alloc(3)
            AM = alloc(36); BM = alloc(36); WAM = alloc(36)
            wb = WW.unsqueeze(1).to_broadcast([P, 3, 12])
            tt(WA.rearrange("p (c b) -> p c b", c=3), a3, wb, OP.mult)
            red(SA, WA.rearrange("p (c b) -> p c b", c=3))
            ts(CA, SA, RWS, OP.mult)
            tt(WA.rearrange("p (c b) -> p c b", c=3), b3, wb, OP.mult)
            red(SA, WA.rearrange("p (c b) -> p c b", c=3))
            ts(CB, SA, RWS, OP.mult)
            tt(AM.rearrange("p (c b) -> p c b", c=3), a3,
               CA.unsqueeze(2).to_broadcast([P, 3, 12]), OP.subtract)
            tt(BM.rearrange("p (c b) -> p c b", c=3), b3,
               CB.unsqueeze(2).to_broadcast([P, 3, 12]), OP.subtract)
            tt(WAM.rearrange("p (c b) -> p c b", c=3),
               AM.rearrange("p (c b) -> p c b", c=3), wb, OP.mult)
            HP = alloc(108); H9 = alloc(9)
            tt(HP.rearrange("p (i j b) -> p i j b", i=3, j=3),
               WAM.rearrange("p (c b) -> p c b", c=3).unsqueeze(2).to_broadcast([P, 3, 3, 12]),
               BM.rearrange("p (c b) -> p c b", c=3).unsqueeze(1).to_broadcast([P, 3, 3, 12]),
               OP.mult)
            red(H9, HP.rearrange("p (i j b) -> p i j b", i=3, j=3))
            if debug:
                cp(dbg_sb["dh9"], H9)
            KP = alloc(27); K9 = alloc(9)
            h3v = H9.rearrange("p (i j) -> p i j", i=3)
            tt(KP.rearrange("p (i l j) -> p i l j", i=3, l=3),
               h3v.unsqueeze(2).to_broadcast([P, 3, 3, 3]),
               h3v.unsqueeze(1).to_broadcast([P, 3, 3, 3]), OP.mult)
            red(K9, KP.rearrange("p (i l j) -> p i l j", i=3, l=3))
            if debug:
                cp(dbg_sb["dk9"], K9)

            # ---- closed-form eigenvalues of K (3x3 sym PSD) ----
            c1_ = lambda i: K9[:, i:i + 1]
            QQ = alloc(1)
            tt(QQ, c1_(0), c1_(4), OP.add)
            tt(QQ, QQ, c1_(8), OP.add)
            ts(QQ, QQ, float(F32(1.0 / 3.0)), OP.mult)
            KD = alloc(3)   # K00-qq, K11-qq, K22-qq
            for di, src_i in enumerate((0, 4, 8)):
                tt(KD[:, di:di + 1], c1_(src_i), QQ, OP.subtract)
            P1 = alloc(1); TTa = alloc(1); TTb = alloc(1)
            tt(P1, c1_(1), c1_(1), OP.mult)
            tt(TTa, c1_(2), c1_(2), OP.mult)
            tt(P1, P1, TTa, OP.add)
            tt(TTa, c1_(5), c1_(5), OP.mult)
            tt(P1, P1, TTa, OP.add)
            P2 = alloc(1)
            tt(P2, KD[:, 0:1], KD[:, 0:1], OP.mult)
            tt(TTa, KD[:, 1:2], KD[:, 1:2], OP.mult)
            tt(P2, P2, TTa, OP.add)
            tt(TTa, KD[:, 2:3], KD[:, 2:3], OP.mult)
            tt(P2, P2, TTa, OP.add)
            ts(TTa, P1, 2.0, OP.mult)
            tt(P2, P2, TTa, OP.add)
            PV = alloc(1); RP = alloc(1)
            ts(PV, P2, float(F32(1.0 / 6.0)), OP.mult)
            act_sqrt(PV, PV)
            ts(TTa, PV, 1e-30, OP.max)
            rcp(RP, TTa)
            BV = alloc(6)   # B00,B11,B22,B01,B02,B12
            for bi_, src in enumerate((KD[:, 0:1], KD[:, 1:2], KD[:, 2:3],
                                       c1_(1), c1_(2), c1_(5))):
                ts(BV[:, bi_:bi_ + 1], src, RP, OP.mult)
            B00, B11, B22 = BV[:, 0:1], BV[:, 1:2], BV[:, 2:3]
            B01, B02, B12 = BV[:, 3:4], BV[:, 4:5], BV[:, 5:6]
            DET = alloc(1); TTc = alloc(1)
            # t1 = B00*(B11*B22 - B12*B12)
            tt(TTa, B11, B22, OP.mult)
            tt(TTb, B12, B12, OP.mult)
            tt(TTa, TTa, TTb, OP.subtract)
            tt(DET, B00, TTa, OP.mult)
            # t2 = B01*(B01*B22 - B12*B02); det = t1 - t2
            tt(TTa, B01, B22, OP.mult)
            tt(TTb, B12, B02, OP.mult)
            tt(TTa, TTa, TTb, OP.subtract)
            tt(TTc, B01, TTa, OP.mult)
            tt(DET, DET, TTc, OP.subtract)
            # t3 = B02*(B01*B12 - B11*B02); det = det + t3
            tt(TTa, B01, B12, OP.mult)
            tt(TTb, B11, B02, OP.mult)
            tt(TTa, TTa, TTb, OP.subtract)
            tt(TTc, B02, TTa, OP.mult)
            tt(DET, DET, TTc, OP.add)
            RV = alloc(1)
            ts(RV, DET, 0.5, OP.mult)
            ts(RV, RV, -1.0, OP.max)
            ts(RV, RV, 1.0, OP.min)
            CC = alloc(1); C2 = alloc(1); C3 = alloc(1)
            FF = alloc(1); FP = alloc(1); RFP = alloc(1)
            mset(CC, 1.0)
            for _ in range(6):
                tt(C2, CC, CC, OP.mult)
                tt(C3, C2, CC, OP.mult)
                ts(FF, C3, 4.0, OP.mult)
                ts(TTa, CC, 3.0, OP.mult)
                tt(FF, FF, TTa, OP.subtract)
                tt(FF, FF, RV, OP.subtract)
                ts(FP, C2, 12.0, OP.mult, 3.0, OP.subtract)
                ts(FP, FP, 1e-6, OP.max)
                rcp(RFP, FP)
                tt(TTa, FF, RFP, OP.mult)
                tt(CC, CC, TTa, OP.subtract)
                ts(CC, CC, 0.5, OP.max)
                ts(CC, CC, 1.0, OP.min)
            SS = alloc(1)
            tt(SS, CC, CC, OP.mult)
            ts(SS, SS, -1.0, OP.mult, 1.0, OP.add)
            ts(SS, SS, 0.0, OP.max)
            act_sqrt(SS, SS)
            LAM1 = alloc(1); LAM2 = alloc(1)
            ts(TTa, PV, 2.0, OP.mult)
            tt(TTa, TTa, CC, OP.mult)
            tt(LAM1, QQ, TTa, OP.add)
            ts(TTa, CC, -0.5, OP.mult)
            ts(TTb, SS, float(F32(np.sqrt(3.0) / 2.0)), OP.mult)
            tt(TTa, TTa, TTb, OP.add)
            ts(TTb, PV, 2.0, OP.mult)
            tt(TTa, TTa, TTb, OP.mult)
            tt(LAM2, QQ, TTa, OP.add)
            if debug:
                cp(dbg_sb["dlam"][:, 0:1], LAM1)
                cp(dbg_sb["dlam"][:, 1:2], LAM2)

            # ---- eigenvectors ----
            AK = alloc(9)
            C1v = alloc(3); C2v = alloc(3); C3v = alloc(3)
            N1 = alloc(1); N2e = alloc(1); N3e = alloc(1)
            MA = alloc(1); MB = alloc(1); MC = alloc(1)
            T3v = alloc(3); NU = alloc(1); NS = alloc(1); RNU = alloc(1)
            U1 = alloc(3); U2 = alloc(3); U3 = alloc(3)

            def eigvec(lam, uout):
                cp(AK, K9)
                for d in range(3):
                    tt(AK[:, 4 * d:4 * d + 1], AK[:, 4 * d:4 * d + 1], lam, OP.subtract)
                r0, r1, r2 = AK[:, 0:3], AK[:, 3:6], AK[:, 6:9]
                cross3(C1v, r0, r1, TTa, TTb)
                cross3(C2v, r1, r2, TTa, TTb)
                cross3(C3v, r2, r0, TTa, TTb)
                for (cv, nv) in ((C1v, N1), (C2v, N2e), (C3v, N3e)):
                    tt(T3v, cv, cv, OP.mult)
                    red(nv, T3v)
                tt(MA, N1, N2e, OP.is_ge)
                tt(TTa, N1, N3e, OP.is_ge)
                tt(MA, MA, TTa, OP.mult)
                ts(TTa, MA, -1.0, OP.mult, 1.0, OP.add)     # 1 - a1
                tt(MB, N2e, N3e, OP.is_ge)
                tt(MB, TTa, MB, OP.mult)                     # a2
                tt(MC, TTa, MB, OP.subtract)                 # a3
                ts(uout, C1v, MA, OP.mult)
                ts(T3v, C2v, MB, OP.mult)
                tt(uout, uout, T3v, OP.add)
                ts(T3v, C3v, MC, OP.mult)
                tt(uout, uout, T3v, OP.add)
                normalize3(uout, NU, NS, RNU, T3v)

            eigvec(LAM1, U1)
            eigvec(LAM2, U2)
            if debug:
                cp(dbg_sb["du1"], U1)
            # Gram-Schmidt u2 against u1
            DOT = alloc(1)
            tt(T3v, U1, U2, OP.mult)
            red(DOT, T3v)
            ts(T3v, U1, DOT, OP.mult)
            tt(U2, U2, T3v, OP.subtract)
            normalize3(U2, NU, NS, RNU, T3v)
            if debug:
                cp(dbg_sb["du2"], U2)
            cross3(U3, U1, U2, TTa, TTb)

            # v_i = normalize(H^T u_i); v3 = v1 x v2
            HP2 = alloc(9)
            V1 = alloc(3); V2 = alloc(3); V3 = alloc(3)
            ht3 = H9.rearrange("p (i j) -> p i j", i=3).transpose([0, 2, 1])
            for (uin, vout) in ((U1, V1), (U2, V2)):
                tt(HP2.rearrange("p (i j) -> p i j", i=3), ht3,
                   uin.unsqueeze(1).to_broadcast([P, 3, 3]), OP.mult)
                red(vout, HP2.rearrange("p (i j) -> p i j", i=3))
                normalize3(vout, NU, NS, RNU, T3v)
            if debug:
                cp(dbg_sb["dv1"], V1)
            cross3(V3, V1, V2, TTa, TTb)

            # R = v1 u1^T + v2 u2^T + v3 u3^T ;  t = cB - R cA
            OP9 = alloc(9)
            tt(R9S.rearrange("p (i j) -> p i j", i=3),
               V1.unsqueeze(2).to_broadcast([P, 3, 3]),
               U1.unsqueeze(1).to_broadcast([P, 3, 3]), OP.mult)
            for (vv_, uu_) in ((V2, U2), (V3, U3)):
                tt(OP9.rearrange("p (i j) -> p i j", i=3),
                   vv_.unsqueeze(2).to_broadcast([P, 3, 3]),
                   uu_.unsqueeze(1).to_broadcast([P, 3, 3]), OP.mult)
                tt(R9S, R9S, OP9, OP.add)
            tt(OP9.rearrange("p (i j) -> p i j", i=3),
               R9S.rearrange("p (i j) -> p i j", i=3),
               CA.unsqueeze(1).to_broadcast([P, 3, 3]), OP.mult)
            RC = alloc(3)
            red(RC, OP9.rearrange("p (i j) -> p i j", i=3))
            tt(T3S, CB, RC, OP.subtract)

            if trunc == 3:
                fence()
                return
            # ---- fitness: count ||R x + t - y|| < 0.1 over all 2048 pts ----
            DC = SCR[:, 0:6144].rearrange("p (c n) -> p c n", c=3)
            ACC = SCR[:, 6144:6144 + 2048]
            L2S = SCR[:, 8192:8192 + 2048]
            SQ = SCR[:, 10240:10240 + 2048]
            xv = PSRC.rearrange("p (c n) -> p c n", c=3)
            yv = PTGT.rearrange("p (c n) -> p c n", c=3)
            for c in range(3):
                ts(ACC, xv[:, 0, :], R9S[:, 3 * c:3 * c + 1], OP.mult,
                   T3S[:, c:c + 1], OP.add)
                stt(ACC, xv[:, 1, :], R9S[:, 3 * c + 1:3 * c + 2], ACC, OP.mult, OP.add)
                stt(ACC, xv[:, 2, :], R9S[:, 3 * c + 2:3 * c + 3], ACC, OP.mult, OP.add)
                tt(DC[:, c, :], ACC, yv[:, c, :], OP.subtract)
            tt(L2S, DC[:, 0, :], DC[:, 0, :], OP.mult)
            tt(SQ, DC[:, 1, :], DC[:, 1, :], OP.mult)
            tt(L2S, L2S, SQ, OP.add)
            tt(SQ, DC[:, 2, :], DC[:, 2, :], OP.mult)
            tt(L2S, L2S, SQ, OP.add)
            ts(SQ, L2S, float(T2), OP.is_lt)
            red(CNTS, SQ)
            fence()

        @block.scalar
        def _(scalar):
            from concourse import mybir as mb
            for (vt, src, dst) in sqrt_jobs:
                scalar.wait_ge(vsem, vt)
                nc.scalar.sqrt(dst, src).then_inc(asem, 1)

        @block.gpsimd
        def _(gpsimd):
            gpsimd.dma_start(INP, inp[:, :]).then_inc(dins, 16)
            gpsimd.iota(POSI, pattern=[[1, 200]], base=0,
                        channel_multiplier=0).then_inc(gsem, 1)
            # rebuild replicated keypoint rows: scatter -> DRAM -> broadcast
            gpsimd.wait_ge(dins, 16)
            gpsimd.dma_start(dscr[0:1, :].rearrange("p (a b) -> p a b", a=P),
                             INP[:, 200:392]).then_inc(dpts, 16)
            gpsimd.wait_ge(dpts, 16)
            gpsimd.dma_start(PSRC, dscr[0:1, 0:3 * NPTS].to_broadcast([P, 3 * NPTS])).then_inc(dpts, 16)
            gpsimd.dma_start(PTGT, dscr[0:1, 3 * NPTS:6 * NPTS].to_broadcast([P, 3 * NPTS])).then_inc(dpts, 16)
            gpsimd.wait_ge(vsem, vcnt[0])
            nout = 1 + len(dbg_names)
            gpsimd.dma_start(out13[:, :], OUT13).then_inc(dout, 16)
            if debug:
                for n_ in dbg_names:
                    gpsimd.dma_start(dbg_dram[n_][:, :], dbg_sb[n_]).then_inc(dout, 16)
            gpsimd.wait_ge(dout, 16 * nout)
    return nc


def _get_prog(key, builder):
    if key not in _programs:
        nc = builder()
        # The program is frozen after build; memoize its BIR serialization so
        # each launch's lowering doesn't re-serialize ~1MB of json (~12ms).
        raw = nc.to_json_bytes()
        nc.to_json_bytes = lambda: raw
        _programs[key] = nc
    return _programs[key]


_cache_cfg = []


def _enable_jax_cache():
    if _cache_cfg:
        return
    _cache_cfg.append(1)
    try:
        import jax
        jax.config.update("jax_compilation_cache_dir", "/tmp/jax_comp_cache")
        jax.config.update("jax_persistent_cache_min_compile_time_secs", 0)
        jax.config.update("jax_persistent_cache_min_entry_size_bytes", 0)
    except Exception:
        pass


def _run(nc, in_maps):
    import time
    _enable_jax_cache()
    from concourse.bass_utils import run_bass_kernel_spmd
    last = None
    for attempt in range(3):
        try:
            t0 = time.time()
            res = run_bass_kernel_spmd(nc, in_maps, core_ids=list(range(NCORES)))
            _launch_wall.append(time.time() - t0)
            return res.results
        except Exception as e:  # transient device errors: retry
            last = e
    raise last


def kernel(SC2_measure, src_keypts, tgt_keypts):
    _launch_wall.clear()
    SC2 = np.ascontiguousarray(SC2_measure[0], dtype=np.float32)      # [512, 2048]
    src = np.ascontiguousarray(src_keypts[0], dtype=np.float32)       # [2048, 3]
    tgt = np.ascontiguousarray(tgt_keypts[0], dtype=np.float32)

    # exact top-200 per seed (desc value, ties -> lower index == lax.top_k)
    knn = np.argsort(-SC2, axis=1, kind='stable')[:, :200]
    knnf = knn.astype(np.float32)                                     # ints < 2048, exact
    ptsflat = np.concatenate([src.T.reshape(3 * NPTS), tgt.T.reshape(3 * NPTS)])
    ptsrows = ptsflat.reshape(SPC, 192)                               # row r: flat[192r:192r+192]

    nc = _get_prog("mega", _prog_mega)
    in_maps = []
    for c in range(NCORES):
        m = np.empty((SPC, 392), np.float32)
        m[:, 0:200] = knnf[c * SPC:(c + 1) * SPC]
        m[:, 200:392] = ptsrows
        in_maps.append({"inp": m})
    for _try in range(3):
        res = _run(nc, in_maps)
        o = np.concatenate([res[c]["out13"] for c in range(NCORES)])  # [512,13]
        cntv = o[:, 0]
        R = o[:, 1:10].reshape(SEEDS, 3, 3)
        t = o[:, 10:13]
        ok = (np.isfinite(cntv).all() and (cntv == np.round(cntv)).all()
              and (cntv >= 0).all() and (cntv <= NPTS).all()
              and np.isfinite(R).all() and np.isfinite(t).all())
        if ok:
            break
    best = int(np.argmax(cntv))
    T = np.zeros((1, 4, 4), np.float32)
    T[0, :3, :3] = R[best]
    T[0, :3, 3] = t[best]
    T[0, 3, 3] = 1.0
    return T
